# revision 14
# baseline (speedup 1.0000x reference)
"""Bass/Trainium2 kernel for nn_CausalGraphVAE (GCN message passing VAE).

Sharding: adjacency columns (= AnT output rows) split across 8 cores.
Per core: sigmoid+deg over its (4096,512) column block of edge_score
(in-place sigmoid, deferred A writes), dis=1/sqrt(deg) via broadcast
matmul, tiny dis AllGather, dis_i folded into the resident bf16 a-tiles,
X-side matmuls sharded by node rows with an early unscaled bf16
AllGather, big matmul A[:,rc]^T @ Ys per core, gates/latent local
(H0=0 kills the R gate; gate biases folded on host), second AllGather
for the decoder, outputs gathered on host. DMA traffic is split across
the two HWDGE rings: sync carries the big edge_score stream + A writes,
scalar carries everything else.
"""
import sys

if "/opt/trn_rl_repo" not in sys.path:
    sys.path.insert(0, "/opt/trn_rl_repo")

import numpy as np
import ml_dtypes

import concourse.bass as bass
import concourse.tile as tile
from concourse import bacc, mybir
from concourse.bass_utils import run_bass_kernel_spmd

NCORES = 8
N = 4096
COLS = N // NCORES          # 512 adjacency columns per core
KT = N // 128               # 32 contraction k-tiles
GSIZE = 4                   # k-tiles per DMA group in the stream
NG = KT // GSIZE            # 4 stream groups
D_IN = 64
D_EMB = 128
HID = 128
LAT = 64
P = 3
YF = P * 2 * HID            # 768 encoder Y features (z|h per period)
KB = 4                      # k-tiles per lhsT DMA batch in big matmuls
F32 = mybir.dt.float32
BF16 = mybir.dt.bfloat16
BF = ml_dtypes.bfloat16

# weight blob layout: name -> (offset, partitions, cols)
WSPEC = [
    ("w_ent", D_EMB, HID), ("w_time", D_EMB, HID),
    ("wc_x", D_IN, 2 * HID), ("wc_e", HID, 2 * HID), ("wc_t", HID, 2 * HID),
    ("wl_z", HID, HID), ("wl_h", HID, HID),
    ("w_mu", HID, LAT), ("w_lv", HID, LAT), ("w_dec", LAT, HID),
    ("wd_cat", HID, 2 * D_IN), ("wld_z", D_IN, D_IN), ("wld_h", D_IN, D_IN),
]
WOFF = {}
_o = 0
for _n, _p, _c in WSPEC:
    WOFF[_n] = (_o, _p, _c)
    _o += _c
WBLOB_COLS = _o

BSPEC = [("nblz", HID, 1), ("blh", HID, 1), ("b_mu", LAT, 1), ("b_lv", LAT, 1),
         ("b_dec", HID, 1), ("nbldz", D_IN, 1), ("bldh", D_IN, 1), ("probs", 128, P),
         ("id4", 4, 4), ("id32", 32, 32)]
BOFF = {}
_o = 0
for _n, _p, _c in BSPEC:
    BOFF[_n] = (_o, _p, _c)
    _o += _c
BBLOB_COLS = _o

_CACHE = {}


def _build():
    nc = bacc.Bacc("TRN2", debug=False, num_devices=NCORES)
    AF = mybir.ActivationFunctionType

    esc = nc.dram_tensor("esc", [128, KT * COLS], F32, kind="ExternalInput")
    xTp = nc.dram_tensor("xTp", [D_IN, P * COLS], BF16, kind="ExternalInput")
    eeTp = nc.dram_tensor("eeTp", [D_EMB, P * COLS], BF16, kind="ExternalInput")
    teTp = nc.dram_tensor("teTp", [D_EMB, P * COLS], BF16, kind="ExternalInput")
    epsT = nc.dram_tensor("epsT", [LAT, COLS], F32, kind="ExternalInput")
    wblob = nc.dram_tensor("wblob", [128, WBLOB_COLS], BF16, kind="ExternalInput")
    bblob = nc.dram_tensor("bblob", [128, BBLOB_COLS], F32, kind="ExternalInput")

    a_out = nc.dram_tensor("a_out", [128, KT * COLS], F32, kind="ExternalOutput")
    mu_out = nc.dram_tensor("mu_out", [LAT, COLS], F32, kind="ExternalOutput")
    lv_out = nc.dram_tensor("lv_out", [LAT, COLS], F32, kind="ExternalOutput")
    rec_out = nc.dram_tensor("rec_out", [D_IN, COLS], F32, kind="ExternalOutput")

    dis_dram = nc.dram_tensor("dis_dram", [1, COLS], F32)
    dis_full = nc.dram_tensor("dis_full", [KT, 128], F32, addr_space="Shared")

    with tile.TileContext(nc) as tc:
        with (
            tc.tile_pool(name="singles", bufs=1) as sg,
            tc.tile_pool(name="esc_in", bufs=8) as esc_p,
            tc.tile_pool(name="ys4", bufs=3) as ys4_p,
            tc.tile_pool(name="yd4", bufs=2) as yd4_p,
            tc.tile_pool(name="work", bufs=1) as wk,
            tc.tile_pool(name="gps", bufs=1, space="PSUM") as gps,
            tc.tile_pool(name="mps", bufs=1, space="PSUM") as mps,
            tc.tile_pool(name="dram", bufs=1, space="DRAM") as dr,
        ):
            # ---- small loads first (scalar ring, blob DMAs) ----
            wblob_t = sg.tile([128, WBLOB_COLS], BF16)
            nc.scalar.dma_start(out=wblob_t[:], in_=wblob[:])
            bblob_t = sg.tile([128, BBLOB_COLS], F32)
            nc.scalar.dma_start(out=bblob_t[:], in_=bblob[:])

            def w(name):
                o, p, c = WOFF[name]
                return wblob_t[0:p, o:o + c]

            def b(name):
                o, p, c = BOFF[name]
                return bblob_t[0:p, o:o + c]

            xT_t = sg.tile([D_IN, P * COLS], BF16)
            nc.scalar.dma_start(out=xT_t[:], in_=xTp[:])
            eeT_t = sg.tile([D_EMB, P * COLS], BF16)
            nc.scalar.dma_start(out=eeT_t[:], in_=eeTp[:])
            teT_t = sg.tile([D_EMB, P * COLS], BF16)
            nc.scalar.dma_start(out=teT_t[:], in_=teTp[:])
            epsT_t = sg.tile([LAT, COLS], F32)
            nc.scalar.dma_start(out=epsT_t[:], in_=epsT[:])
            ones_t = sg.tile([128, 1], BF16)
            nc.vector.memset(ones_t[:], 1.0)
            ones_row = sg.tile([1, 128], F32)
            nc.vector.memset(ones_row[:], 1.0)

            # ---- ent/tim features first (relu on DVE to avoid ACT FIFO) ----
            ent_t = sg.tile([HID, P * COLS], BF16)
            tim_t = sg.tile([HID, P * COLS], BF16)
            for p in range(P):
                psl = slice(p * COLS, (p + 1) * COLS)
                ps1 = gps.tile([HID, COLS], F32, tag="g2", name="ent_ps")
                nc.tensor.matmul(ps1[:], w("w_ent"), eeT_t[:, psl], start=True, stop=True)
                nc.vector.tensor_scalar_max(ent_t[:, psl], ps1[:], 0.0)
                ps2 = gps.tile([HID, COLS], F32, tag="g3", name="tim_ps")
                nc.tensor.matmul(ps2[:], w("w_time"), teT_t[:, psl], start=True, stop=True)
                nc.vector.tensor_scalar_max(tim_t[:, psl], ps2[:], 0.0)

            # ---- local Y shard (node-major, UNSCALED bf16), fire AG1 early ----
            ag1_in = dr.tile([COLS, YF], BF16)
            ag1_out = dr.tile([N, YF], BF16, addr_space="Shared")
            MT = COLS // 128
            for p in range(P):
                for m in range(MT):
                    msl = slice(m * 128, (m + 1) * 128)
                    psl = slice(p * COLS, (p + 1) * COLS)
                    y_ps = gps.tile([128, 2 * HID], F32, tag=f"g{m % 2}", name="y_ps")
                    nc.tensor.matmul(y_ps[:], xT_t[:, psl][:, msl], w("wc_x"),
                                     start=True, stop=False)
                    nc.tensor.matmul(y_ps[:], ent_t[:, psl][:, msl], w("wc_e"),
                                     start=False, stop=False)
                    nc.tensor.matmul(y_ps[:], tim_t[:, psl][:, msl], w("wc_t"),
                                     start=False, stop=True)
                    ysc = wk.tile([128, 2 * HID], BF16, name="ysc")
                    nc.vector.tensor_copy(out=ysc[:], in_=y_ps[:])
                    nc.scalar.dma_start(
                        out=ag1_in[m * 128:(m + 1) * 128, p * 2 * HID:(p + 1) * 2 * HID],
                        in_=ysc[:])
            nc.gpsimd.collective_compute(
                "AllGather", mybir.AluOpType.bypass,
                ins=[ag1_in[:].opt()], outs=[ag1_out[:].opt()],
                replica_groups=[list(range(NCORES))],
            )

            # ---- stream: esc -> sigmoid (in place) -> bf16 cast -> deg ----
            deg_ps = mps.tile([1, COLS], F32, tag="uda", name="deg_ps")
            esc_ts = []
            a_bf = []
            W = GSIZE * COLS
            for g in range(NG):
                esc_t = esc_p.tile([128, W], F32, name="esc_t")
                ring = nc.sync if g % 2 == 0 else nc.scalar
                ring.dma_start(out=esc_t[:], in_=esc[:, g * W:(g + 1) * W])
                nc.scalar.activation(out=esc_t[:], in_=esc_t[:], func=AF.Sigmoid)
                esc_ts.append(esc_t)
                ab = sg.tile([128, W], BF16, name=f"a_bf{g}")
                nc.scalar.activation(out=ab[:], in_=esc_t[:],
                                     func=AF.Copy)
                a_bf.append(ab)
                for kk in range(GSIZE):
                    nc.tensor.matmul(
                        deg_ps[:], ones_t[:], ab[:, kk * COLS:(kk + 1) * COLS],
                        start=(g == 0 and kk == 0), stop=(g == NG - 1 and kk == GSIZE - 1),
                    )

            def a_tile(ki):
                return a_bf[ki // GSIZE][:, (ki % GSIZE) * COLS:(ki % GSIZE + 1) * COLS]

            # ---- dis = 1/sqrt(deg): row chain, broadcast, tiny AllGather ----
            deg_sb = sg.tile([1, COLS], F32)
            nc.vector.tensor_copy(out=deg_sb[:], in_=deg_ps[:])
            sq_row = sg.tile([1, COLS], F32)
            nc.scalar.activation(out=sq_row[:], in_=deg_sb[:], func=AF.Sqrt)
            dis_row = sg.tile([1, COLS], F32)
            rscr = sg.tile([1, COLS], F32)
            nc.vector.reciprocal_approx_accurate(out=dis_row[:], in_=sq_row[:], scratch=rscr[:])
            bc_ps = mps.tile([128, COLS], F32, tag="sp", name="bc_ps")
            nc.tensor.matmul(bc_ps[:], ones_row[:], dis_row[:], start=True, stop=True)
            dis_bc = sg.tile([128, COLS], F32)
            nc.vector.tensor_copy(out=dis_bc[:], in_=bc_ps[:])
            nc.gpsimd.dma_start(out=dis_dram[:], in_=dis_row[:])
            nc.gpsimd.collective_compute(
                "AllGather", mybir.AluOpType.bypass,
                ins=[dis_dram[:].opt()], outs=[dis_full[:].opt()],
                replica_groups=[list(range(NCORES))],
            )
            disf_32 = sg.tile([KT, 128], F32)
            nc.scalar.dma_start(out=disf_32[:], in_=dis_full[:])
            tp_ps = mps.tile([128, KT], F32, tag="sp", name="tp_ps")
            nc.tensor.transpose(tp_ps[:], disf_32[:], b("id32"))
            disf_nm = sg.tile([128, KT], F32)
            nc.vector.tensor_copy(out=disf_nm[:], in_=tp_ps[:])

            # ---- fold dis_i into the resident a-tiles (serves enc + dec) ----
            for ki in range(KT):
                at = a_tile(ki)
                nc.vector.tensor_scalar_mul(at, at, disf_nm[:, ki:ki + 1])

            # ---- deferred A output writes (data-dep gated on deg) ----
            gate_t = sg.tile([1, 1], F32)
            nc.vector.tensor_scalar(gate_t[:], deg_sb[0:1, 0:1], 0.0, 1.0,
                                    mybir.AluOpType.mult, mybir.AluOpType.add)
            for g in range(NG):
                nc.vector.tensor_scalar_mul(esc_ts[g][0:1, 0:1], esc_ts[g][0:1, 0:1],
                                            gate_t[0:1, 0:1])
                nc.sync.dma_start(out=a_out[:, g * W:(g + 1) * W], in_=esc_ts[g][:])

            # ---- encoder big matmul ----
            g_ps = [gps.tile([128, COLS], F32, tag=f"g{ft}", name=f"g_ps{ft}")
                    for ft in range(6)]
            for kb in range(KT // KB):
                ys4 = ys4_p.tile([128, KB, YF], BF16, name="ys4")
                nc.scalar.dma_start(
                    out=ys4[:],
                    in_=ag1_out[kb * KB * 128:(kb + 1) * KB * 128, :]
                    .rearrange("(b p) f -> p b f", p=128))
                for kk in range(KB):
                    ki = kb * KB + kk
                    for ft in range(6):
                        nc.tensor.matmul(
                            g_ps[ft][:], ys4[:, kk, ft * 128:(ft + 1) * 128], a_tile(ki),
                            start=(ki == 0), stop=(ki == KT - 1))

            # ---- encoder gates + Henc ----
            henc_t = sg.tile([HID, COLS], F32)
            for p in range(P):
                gz_sc = wk.tile([128, COLS], BF16, name="gz_sc")
                nc.vector.tensor_mul(gz_sc[:], g_ps[2 * p][:], dis_bc[:])
                u_ps = mps.tile([128, COLS], F32, tag="uda", name="uz_ps")
                nc.tensor.matmul(u_ps[:], w("wl_z"), gz_sc[:], start=True, stop=True)
                zc_t = wk.tile([HID, COLS], F32, name="zc_t")
                nc.scalar.activation(out=zc_t[:], in_=u_ps[:], func=AF.Sigmoid,
                                     bias=b("nblz"), scale=-1.0)
                gh_sc = wk.tile([128, COLS], BF16, name="gh_sc")
                nc.vector.tensor_mul(gh_sc[:], g_ps[2 * p + 1][:], dis_bc[:])
                uh_ps = mps.tile([128, COLS], F32, tag="uda", name="uh_ps")
                nc.tensor.matmul(uh_ps[:], w("wl_h"), gh_sc[:], start=True, stop=True)
                ht_t = wk.tile([HID, COLS], F32, name="ht_t")
                nc.scalar.activation(out=ht_t[:], in_=uh_ps[:], func=AF.Tanh,
                                     bias=b("blh"))
                zh_t = wk.tile([HID, COLS], F32, name="zh_t")
                nc.vector.tensor_mul(zh_t[:], zc_t[:], ht_t[:])
                if p == 0:
                    nc.vector.tensor_scalar_mul(henc_t[:], zh_t[:], b("probs")[:, 0:1])
                else:
                    zhp_t = wk.tile([HID, COLS], F32, name="zhp_t")
                    nc.vector.tensor_scalar_mul(zhp_t[:], zh_t[:], b("probs")[:, p:p + 1])
                    nc.vector.tensor_add(henc_t[:], henc_t[:], zhp_t[:])

            # ---- latent head ----
            h_bf = sg.tile([HID, COLS], BF16)
            nc.scalar.activation(out=h_bf[:], in_=henc_t[:], func=AF.Relu)
            mu_ps = mps.tile([LAT, COLS], F32, tag="sp", name="mu_ps")
            nc.tensor.matmul(mu_ps[:], w("w_mu"), h_bf[:], start=True, stop=True)
            mu_t = sg.tile([LAT, COLS], F32)
            nc.vector.tensor_scalar_add(mu_t[:], mu_ps[:], b("b_mu"))
            nc.scalar.dma_start(out=mu_out[:], in_=mu_t[:])
            lv_ps = mps.tile([LAT, COLS], F32, tag="sp", name="lv_ps")
            nc.tensor.matmul(lv_ps[:], w("w_lv"), h_bf[:], start=True, stop=True)
            lv_t = sg.tile([LAT, COLS], F32)
            nc.vector.tensor_scalar_add(lv_t[:], lv_ps[:], b("b_lv"))
            nc.scalar.dma_start(out=lv_out[:], in_=lv_t[:])
            std_t = wk.tile([LAT, COLS], F32, name="std_t")
            nc.scalar.activation(out=std_t[:], in_=lv_t[:], func=AF.Exp, scale=0.5)
            es_t = wk.tile([LAT, COLS], F32, name="es_t")
            nc.vector.tensor_mul(es_t[:], epsT_t[:], std_t[:])
            z_bf = sg.tile([LAT, COLS], BF16)
            nc.vector.tensor_add(z_bf[:], mu_t[:], es_t[:])
            d_ps = mps.tile([HID, COLS], F32, tag="sp", name="d_ps")
            nc.tensor.matmul(d_ps[:], w("w_dec"), z_bf[:], start=True, stop=True)
            d_bf = sg.tile([HID, COLS], BF16)
            nc.vector.tensor_scalar_add(d_bf[:], d_ps[:], b("b_dec"))

            # ---- decoder Y shard (unscaled) + AllGather ----
            ag2_in = dr.tile([COLS, 2 * D_IN], BF16)
            ag2_out = dr.tile([N, 2 * D_IN], BF16, addr_space="Shared")
            for m in range(MT):
                yd_ps = gps.tile([128, 2 * D_IN], F32, tag=f"g{2 + m % 2}", name="yd_ps")
                nc.tensor.matmul(yd_ps[:], d_bf[:, m * 128:(m + 1) * 128], w("wd_cat"),
                                 start=True, stop=True)
                ydsc = wk.tile([128, 2 * D_IN], BF16, name="ydsc")
                nc.vector.tensor_copy(out=ydsc[:], in_=yd_ps[:])
                nc.scalar.dma_start(out=ag2_in[m * 128:(m + 1) * 128, :], in_=ydsc[:])
            nc.gpsimd.collective_compute(
                "AllGather", mybir.AluOpType.bypass,
                ins=[ag2_in[:].opt()], outs=[ag2_out[:].opt()],
                replica_groups=[list(range(NCORES))],
            )

            # ---- decoder big matmul (two M=64 gates) + gates + recon ----
            gdz_ps = gps.tile([D_IN, COLS], F32, tag="g0", name="gdz_ps")
            gdh_ps = gps.tile([D_IN, COLS], F32, tag="g1", name="gdh_ps")
            for kb in range(KT // KB):
                yd4 = yd4_p.tile([128, KB, 2 * D_IN], BF16, name="yd4")
                nc.scalar.dma_start(
                    out=yd4[:],
                    in_=ag2_out[kb * KB * 128:(kb + 1) * KB * 128, :]
                    .rearrange("(b p) f -> p b f", p=128))
                for kk in range(KB):
                    ki = kb * KB + kk
                    nc.tensor.matmul(gdz_ps[:], yd4[:, kk, 0:D_IN], a_tile(ki),
                                     start=(ki == 0), stop=(ki == KT - 1))
                    nc.tensor.matmul(gdh_ps[:], yd4[:, kk, D_IN:2 * D_IN], a_tile(ki),
                                     start=(ki == 0), stop=(ki == KT - 1))
            gdz_sc = wk.tile([D_IN, COLS], BF16, name="gdz_sc")
            nc.vector.tensor_mul(gdz_sc[:], gdz_ps[:], dis_bc[0:D_IN, :])
            uzd_ps = mps.tile([D_IN, COLS], F32, tag="uda", name="uzd_ps")
            nc.tensor.matmul(uzd_ps[:], w("wld_z"), gdz_sc[:], start=True, stop=True)
            zcd_t = wk.tile([D_IN, COLS], F32, name="zcd_t")
            nc.scalar.activation(out=zcd_t[:], in_=uzd_ps[:], func=AF.Sigmoid,
                                 bias=b("nbldz"), scale=-1.0)
            gdh_sc = wk.tile([D_IN, COLS], BF16, name="gdh_sc")
            nc.vector.tensor_mul(gdh_sc[:], gdh_ps[:], dis_bc[0:D_IN, :])
            uhd_ps = mps.tile([D_IN, COLS], F32, tag="uda", name="uhd_ps")
            nc.tensor.matmul(uhd_ps[:], w("wld_h"), gdh_sc[:], start=True, stop=True)
            htd_t = wk.tile([D_IN, COLS], F32, name="htd_t")
            nc.scalar.activation(out=htd_t[:], in_=uhd_ps[:], func=AF.Tanh,
                                 bias=b("bldh"))
            prod_t = wk.tile([D_IN, COLS], F32, name="prod_t")
            nc.vector.tensor_mul(prod_t[:], zcd_t[:], htd_t[:])
            rec_t = wk.tile([D_IN, COLS], F32, name="rec_t")
            nc.vector.tensor_scalar_max(rec_t[:], prod_t[:], 0.0)
            nc.scalar.dma_start(out=rec_out[:], in_=rec_t[:])

    nc.compile()
    return nc


def _get_nc():
    if "nc" not in _CACHE:
        _CACHE["nc"] = _build()
    return _CACHE["nc"]


def _eps():
    if "eps" not in _CACHE:
        import jax

        with jax.default_device(jax.devices("cpu")[0]):
            e = jax.random.normal(jax.random.key(42), (N, LAT), jax.numpy.float32)
        _CACHE["eps"] = np.asarray(e)
    return _CACHE["eps"]


def _np(v):
    return np.asarray(v, dtype=np.float32)


def _pack_T(arr_rc, feat):
    # (COLS, P, feat) -> (feat, P*COLS), period-major column blocks, bf16
    a = arr_rc.transpose(1, 2, 0)  # (P, feat, COLS)
    out = np.empty((feat, P * COLS), dtype=BF)
    for p in range(P):
        out[:, p * COLS:(p + 1) * COLS] = a[p].astype(BF)
    return out


def make_in_maps(x, entity_emb, time_emb, params):
    x = _np(x)
    ee = _np(entity_emb)
    te = _np(time_emb)
    p = params
    t1, td = p["t1"], p["td"]
    eps = _eps()

    wc = np.concatenate([_np(t1["Wc_z"]), _np(t1["Wc_h"])], 1)
    wvals = {
        "w_ent": _np(p["W_ent"]), "w_time": _np(p["W_time"]),
        "wc_x": wc[:D_IN], "wc_e": wc[D_IN:D_IN + HID], "wc_t": wc[D_IN + HID:],
        "wl_z": _np(t1["Wl_z"])[:HID], "wl_h": _np(t1["Wl_h"])[:HID],
        "w_mu": _np(p["W_mu"]), "w_lv": _np(p["W_lv"]), "w_dec": _np(p["W_dec"]),
        "wd_cat": np.concatenate([_np(td["Wc_z"]), _np(td["Wc_h"])], 1),
        "wld_z": _np(td["Wl_z"])[:D_IN], "wld_h": _np(td["Wl_h"])[:D_IN],
    }
    wblob = np.zeros((128, WBLOB_COLS), dtype=BF)
    for name, (o, pp, c) in WOFF.items():
        wblob[0:pp, o:o + c] = wvals[name].astype(BF)

    att = _np(p["att1"])
    pr = np.exp(att - att.max())
    pr = (pr / pr.sum()).astype(np.float32)
    bvals = {
        "nblz": -(_np(t1["bc_z"]) @ _np(t1["Wl_z"])[:HID] + _np(t1["bl_z"])).reshape(HID, 1),
        "blh": (_np(t1["bc_h"]) @ _np(t1["Wl_h"])[:HID] + _np(t1["bl_h"])).reshape(HID, 1),
        "b_mu": _np(p["b_mu"]).reshape(LAT, 1),
        "b_lv": _np(p["b_lv"]).reshape(LAT, 1),
        "b_dec": _np(p["b_dec"]).reshape(HID, 1),
        "nbldz": -(_np(td["bc_z"]) @ _np(td["Wl_z"])[:D_IN] + _np(td["bl_z"])).reshape(D_IN, 1),
        "bldh": (_np(td["bc_h"]) @ _np(td["Wl_h"])[:D_IN] + _np(td["bl_h"])).reshape(D_IN, 1),
        "probs": np.broadcast_to(pr, (128, P)),
        "id4": np.eye(4, dtype=np.float32),
        "id32": np.eye(32, dtype=np.float32),
    }
    bblob = np.zeros((128, BBLOB_COLS), dtype=np.float32)
    for name, (o, pp, c) in BOFF.items():
        bblob[0:pp, o:o + c] = bvals[name].astype(np.float32)

    es_full = _np(p["edge_score"])
    in_maps = []
    for c in range(NCORES):
        rc = slice(c * COLS, (c + 1) * COLS)
        blk = es_full[:, rc]  # (4096, 512)
        esc_tiled = np.ascontiguousarray(
            blk.reshape(KT, 128, COLS).transpose(1, 0, 2).reshape(128, KT * COLS))
        in_maps.append({
            "wblob": wblob, "bblob": bblob, "esc": esc_tiled,
            "xTp": _pack_T(x[rc], D_IN),
            "eeTp": _pack_T(ee[rc], D_EMB),
            "teTp": _pack_T(te[rc], D_EMB),
            "epsT": np.ascontiguousarray(eps[rc].T),
        })
    return in_maps


def assemble(results):
    a_blocks, mu_blocks, lv_blocks, rec_blocks = [], [], [], []
    for c in range(NCORES):
        r = results[c]
        a_blocks.append(
            r["a_out"].reshape(128, KT, COLS).transpose(1, 0, 2).reshape(N, COLS))
        mu_blocks.append(r["mu_out"].T)
        lv_blocks.append(r["lv_out"].T)
        rec_blocks.append(r["rec_out"].T)
    A = np.concatenate(a_blocks, axis=1)
    mu = np.concatenate(mu_blocks, axis=0)
    lv = np.concatenate(lv_blocks, axis=0)
    rec = np.concatenate(rec_blocks, axis=0)
    return rec, mu, lv, A


def kernel(x, entity_emb, time_emb, num_nodes, params):
    nc = _get_nc()
    in_maps = make_in_maps(x, entity_emb, time_emb, params)
    res = run_bass_kernel_spmd(nc, in_maps, list(range(NCORES)))
    return assemble(res.results)


# revision 15
# speedup vs baseline: 1.0039x; 1.0039x over previous
"""Bass/Trainium2 kernel for nn_CausalGraphVAE (GCN message passing VAE).

Sharding: adjacency columns (= AnT output rows) split across 8 cores.
Per core: sigmoid+deg over its (4096,512) column block of edge_score
(in-place sigmoid, deferred A writes), dis=1/sqrt(deg) via broadcast
matmul, tiny dis AllGather, dis_i folded into the resident bf16 a-tiles,
X-side matmuls sharded by node rows with an early unscaled bf16
AllGather, big matmul A[:,rc]^T @ Ys per core, gates/latent local
(H0=0 kills the R gate; gate biases folded on host), second AllGather
for the decoder, outputs gathered on host. DMA traffic is split across
the two HWDGE rings: sync carries the big edge_score stream + A writes,
scalar carries everything else.
"""
import sys

if "/opt/trn_rl_repo" not in sys.path:
    sys.path.insert(0, "/opt/trn_rl_repo")

import numpy as np
import ml_dtypes

import concourse.bass as bass
import concourse.tile as tile
from concourse import bacc, mybir
from concourse.bass_utils import run_bass_kernel_spmd

NCORES = 8
N = 4096
COLS = N // NCORES          # 512 adjacency columns per core
KT = N // 128               # 32 contraction k-tiles
GSIZE = 4                   # k-tiles per DMA group in the stream
NG = KT // GSIZE            # 4 stream groups
D_IN = 64
D_EMB = 128
HID = 128
LAT = 64
P = 3
YF = P * 2 * HID            # 768 encoder Y features (z|h per period)
KB = 4                      # k-tiles per lhsT DMA batch in big matmuls
F32 = mybir.dt.float32
BF16 = mybir.dt.bfloat16
BF = ml_dtypes.bfloat16

# weight blob layout: name -> (offset, partitions, cols)
WSPEC = [
    ("w_ent", D_EMB, HID), ("w_time", D_EMB, HID),
    ("wc_x", D_IN, 2 * HID), ("wc_e", HID, 2 * HID), ("wc_t", HID, 2 * HID),
    ("wl_z", HID, HID), ("wl_h", HID, HID),
    ("w_mu", HID, LAT), ("w_lv", HID, LAT), ("w_dec", LAT, HID),
    ("wd_cat", HID, 2 * D_IN), ("wld_z", D_IN, D_IN), ("wld_h", D_IN, D_IN),
]
WOFF = {}
_o = 0
for _n, _p, _c in WSPEC:
    WOFF[_n] = (_o, _p, _c)
    _o += _c
WBLOB_COLS = _o

BSPEC = [("nblz", HID, 1), ("blh", HID, 1), ("b_mu", LAT, 1), ("b_lv", LAT, 1),
         ("b_dec", HID, 1), ("nbldz", D_IN, 1), ("bldh", D_IN, 1), ("probs", 128, P),
         ("id4", 4, 4), ("id32", 32, 32)]
BOFF = {}
_o = 0
for _n, _p, _c in BSPEC:
    BOFF[_n] = (_o, _p, _c)
    _o += _c
BBLOB_COLS = _o

_CACHE = {}


def _build():
    nc = bacc.Bacc("TRN2", debug=False, num_devices=NCORES)
    AF = mybir.ActivationFunctionType

    esc = nc.dram_tensor("esc", [128, KT * COLS], F32, kind="ExternalInput")
    xTp = nc.dram_tensor("xTp", [D_IN, P * COLS], BF16, kind="ExternalInput")
    eeTp = nc.dram_tensor("eeTp", [D_EMB, P * COLS], BF16, kind="ExternalInput")
    teTp = nc.dram_tensor("teTp", [D_EMB, P * COLS], BF16, kind="ExternalInput")
    epsT = nc.dram_tensor("epsT", [LAT, COLS], F32, kind="ExternalInput")
    wblob = nc.dram_tensor("wblob", [128, WBLOB_COLS], BF16, kind="ExternalInput")
    bblob = nc.dram_tensor("bblob", [128, BBLOB_COLS], F32, kind="ExternalInput")

    a_out = nc.dram_tensor("a_out", [128, KT * COLS], F32, kind="ExternalOutput")
    mu_out = nc.dram_tensor("mu_out", [LAT, COLS], F32, kind="ExternalOutput")
    lv_out = nc.dram_tensor("lv_out", [LAT, COLS], F32, kind="ExternalOutput")
    rec_out = nc.dram_tensor("rec_out", [D_IN, COLS], F32, kind="ExternalOutput")

    dis_dram = nc.dram_tensor("dis_dram", [1, COLS], F32)
    dis_full = nc.dram_tensor("dis_full", [KT, 128], F32, addr_space="Shared")

    with tile.TileContext(nc) as tc:
        with (
            tc.tile_pool(name="singles", bufs=1) as sg,
            tc.tile_pool(name="esc_in", bufs=8) as esc_p,
            tc.tile_pool(name="ys4", bufs=3) as ys4_p,
            tc.tile_pool(name="yd4", bufs=2) as yd4_p,
            tc.tile_pool(name="work", bufs=1) as wk,
            tc.tile_pool(name="gps", bufs=1, space="PSUM") as gps,
            tc.tile_pool(name="mps", bufs=1, space="PSUM") as mps,
            tc.tile_pool(name="dram", bufs=1, space="DRAM") as dr,
        ):
            # ---- small loads first (scalar ring, blob DMAs) ----
            wblob_t = sg.tile([128, WBLOB_COLS], BF16)
            nc.scalar.dma_start(out=wblob_t[:], in_=wblob[:])
            bblob_t = sg.tile([128, BBLOB_COLS], F32)
            nc.scalar.dma_start(out=bblob_t[:], in_=bblob[:])

            def w(name):
                o, p, c = WOFF[name]
                return wblob_t[0:p, o:o + c]

            def b(name):
                o, p, c = BOFF[name]
                return bblob_t[0:p, o:o + c]

            xT_t = sg.tile([D_IN, P * COLS], BF16)
            nc.scalar.dma_start(out=xT_t[:], in_=xTp[:])
            eeT_t = sg.tile([D_EMB, P * COLS], BF16)
            nc.scalar.dma_start(out=eeT_t[:], in_=eeTp[:])
            teT_t = sg.tile([D_EMB, P * COLS], BF16)
            nc.scalar.dma_start(out=teT_t[:], in_=teTp[:])
            epsT_t = sg.tile([LAT, COLS], F32)
            nc.scalar.dma_start(out=epsT_t[:], in_=epsT[:])
            ones_t = sg.tile([128, 1], BF16)
            nc.vector.memset(ones_t[:], 1.0)
            ones_row = sg.tile([1, 128], F32)
            nc.vector.memset(ones_row[:], 1.0)

            # ---- ent/tim features first (relu on DVE to avoid ACT FIFO) ----
            ent_t = sg.tile([HID, P * COLS], BF16)
            tim_t = sg.tile([HID, P * COLS], BF16)
            for p in range(P):
                psl = slice(p * COLS, (p + 1) * COLS)
                ps1 = gps.tile([HID, COLS], F32, tag="g2", name="ent_ps")
                nc.tensor.matmul(ps1[:], w("w_ent"), eeT_t[:, psl], start=True, stop=True)
                nc.vector.tensor_scalar_max(ent_t[:, psl], ps1[:], 0.0)
                ps2 = gps.tile([HID, COLS], F32, tag="g3", name="tim_ps")
                nc.tensor.matmul(ps2[:], w("w_time"), teT_t[:, psl], start=True, stop=True)
                nc.vector.tensor_scalar_max(tim_t[:, psl], ps2[:], 0.0)

            # ---- local Y shard: drain psum to f32 SBUF early ----
            MT = COLS // 128
            y_sb = []
            for p in range(P):
                for m in range(MT):
                    msl = slice(m * 128, (m + 1) * 128)
                    psl = slice(p * COLS, (p + 1) * COLS)
                    y_ps = gps.tile([128, 2 * HID], F32, tag=f"g{m % 2}", name="y_ps")
                    nc.tensor.matmul(y_ps[:], xT_t[:, psl][:, msl], w("wc_x"),
                                     start=True, stop=False)
                    nc.tensor.matmul(y_ps[:], ent_t[:, psl][:, msl], w("wc_e"),
                                     start=False, stop=False)
                    nc.tensor.matmul(y_ps[:], tim_t[:, psl][:, msl], w("wc_t"),
                                     start=False, stop=True)
                    ysb = sg.tile([128, 2 * HID], F32, name=f"ysb{p}_{m}")
                    nc.vector.tensor_copy(out=ysb[:], in_=y_ps[:])
                    y_sb.append((p, m, ysb))

            # ---- stream: esc -> sigmoid (in place) -> bf16 cast -> deg ----
            deg_ps = mps.tile([1, COLS], F32, tag="uda", name="deg_ps")
            esc_ts = []
            a_bf = []
            W = GSIZE * COLS
            for g in range(NG):
                esc_t = esc_p.tile([128, W], F32, name="esc_t")
                ring = nc.sync if g % 2 == 0 else nc.scalar
                ring.dma_start(out=esc_t[:], in_=esc[:, g * W:(g + 1) * W])
                nc.scalar.activation(out=esc_t[:], in_=esc_t[:], func=AF.Sigmoid)
                esc_ts.append(esc_t)
                ab = sg.tile([128, W], BF16, name=f"a_bf{g}")
                nc.vector.tensor_copy(out=ab[:], in_=esc_t[:])
                a_bf.append(ab)
                for kk in range(GSIZE):
                    nc.tensor.matmul(
                        deg_ps[:], ones_t[:], ab[:, kk * COLS:(kk + 1) * COLS],
                        start=(g == 0 and kk == 0), stop=(g == NG - 1 and kk == GSIZE - 1),
                    )

            def a_tile(ki):
                return a_bf[ki // GSIZE][:, (ki % GSIZE) * COLS:(ki % GSIZE + 1) * COLS]

            # ---- dis = 1/sqrt(deg): row chain, broadcast, tiny AllGather ----
            deg_sb = sg.tile([1, COLS], F32)
            nc.vector.tensor_copy(out=deg_sb[:], in_=deg_ps[:])
            sq_row = sg.tile([1, COLS], F32)
            nc.scalar.activation(out=sq_row[:], in_=deg_sb[:], func=AF.Sqrt)
            dis_row = sg.tile([1, COLS], F32)
            rscr = sg.tile([1, COLS], F32)
            nc.vector.reciprocal_approx_accurate(out=dis_row[:], in_=sq_row[:], scratch=rscr[:])
            bc_ps = mps.tile([128, COLS], F32, tag="sp", name="bc_ps")
            nc.tensor.matmul(bc_ps[:], ones_row[:], dis_row[:], start=True, stop=True)
            dis_bc = sg.tile([128, COLS], F32)
            nc.vector.tensor_copy(out=dis_bc[:], in_=bc_ps[:])
            nc.gpsimd.dma_start(out=dis_dram[:], in_=dis_row[:])
            dis4 = sg.tile([MT, 128], F32)
            nc.gpsimd.dma_start(
                out=dis4[:], in_=dis_dram[0, :].rearrange("(m p) -> m p", m=MT))
            tp_ps = mps.tile([128, MT], F32, tag="sp", name="tp_ps")
            nc.tensor.transpose(tp_ps[:], dis4[:], b("id4"))
            dis_nm = sg.tile([128, MT], F32)
            nc.vector.tensor_copy(out=dis_nm[:], in_=tp_ps[:])

            # ---- scale Y shard rows by dis_i, ship, AllGather in two halves ----
            ag1_in = dr.tile([COLS, YF], BF16)
            ag1_outA = dr.tile([N // 2, YF], BF16, addr_space="Shared")
            ag1_outB = dr.tile([N // 2, YF], BF16, addr_space="Shared")
            for p, m, ysb in y_sb:
                ysc = wk.tile([128, 2 * HID], BF16, name="ysc")
                nc.vector.tensor_scalar_mul(ysc[:], ysb[:], dis_nm[:, m:m + 1])
                nc.scalar.dma_start(
                    out=ag1_in[m * 128:(m + 1) * 128, p * 2 * HID:(p + 1) * 2 * HID],
                    in_=ysc[:])
            nc.gpsimd.collective_compute(
                "AllGather", mybir.AluOpType.bypass,
                ins=[ag1_in[0:COLS // 2, :].opt()], outs=[ag1_outA[:].opt()],
                replica_groups=[list(range(NCORES))],
            )
            nc.gpsimd.collective_compute(
                "AllGather", mybir.AluOpType.bypass,
                ins=[ag1_in[COLS // 2:COLS, :].opt()], outs=[ag1_outB[:].opt()],
                replica_groups=[list(range(NCORES))],
            )

            # ---- deferred A output writes (data-dep gated on deg) ----
            gate_t = sg.tile([1, 1], F32)
            nc.vector.tensor_scalar(gate_t[:], deg_sb[0:1, 0:1], 0.0, 1.0,
                                    mybir.AluOpType.mult, mybir.AluOpType.add)
            for g in range(NG):
                nc.vector.tensor_scalar_mul(esc_ts[g][0:1, 0:1], esc_ts[g][0:1, 0:1],
                                            gate_t[0:1, 0:1])
                nc.sync.dma_start(out=a_out[:, g * W:(g + 1) * W], in_=esc_ts[g][:])

            # ---- encoder big matmul ----
            g_ps = [gps.tile([128, COLS], F32, tag=f"g{ft}", name=f"g_ps{ft}")
                    for ft in range(6)]
            ki_order = []
            for half in (0, 1):
                for c in range(NCORES):
                    for t in (0, 1):
                        ki_order.append((c * 4 + half * 2 + t, half))
            first_ki = ki_order[0][0]
            last_ki = ki_order[-1][0]
            for idx in range(0, len(ki_order), KB):
                batch = ki_order[idx:idx + KB]
                half = batch[0][1]
                buf = ag1_outA if half == 0 else ag1_outB
                row0 = (idx % 16)  # row offset within this half's buffer
                ys4 = ys4_p.tile([128, KB, YF], BF16, name="ys4")
                nc.scalar.dma_start(
                    out=ys4[:],
                    in_=buf[row0 * 128:(row0 + KB) * 128, :]
                    .rearrange("(b p) f -> p b f", p=128))
                for j, (ki, _) in enumerate(batch):
                    for ft in range(6):
                        nc.tensor.matmul(
                            g_ps[ft][:], ys4[:, j, ft * 128:(ft + 1) * 128], a_tile(ki),
                            start=(ki == first_ki), stop=(ki == last_ki))

            # ---- encoder gates + Henc ----
            henc_t = sg.tile([HID, COLS], F32)
            for p in range(P):
                gz_sc = wk.tile([128, COLS], BF16, name="gz_sc")
                nc.vector.tensor_mul(gz_sc[:], g_ps[2 * p][:], dis_bc[:])
                u_ps = mps.tile([128, COLS], F32, tag="uda", name="uz_ps")
                nc.tensor.matmul(u_ps[:], w("wl_z"), gz_sc[:], start=True, stop=True)
                zc_t = wk.tile([HID, COLS], F32, name="zc_t")
                nc.scalar.activation(out=zc_t[:], in_=u_ps[:], func=AF.Sigmoid,
                                     bias=b("nblz"), scale=-1.0)
                gh_sc = wk.tile([128, COLS], BF16, name="gh_sc")
                nc.vector.tensor_mul(gh_sc[:], g_ps[2 * p + 1][:], dis_bc[:])
                uh_ps = mps.tile([128, COLS], F32, tag="uda", name="uh_ps")
                nc.tensor.matmul(uh_ps[:], w("wl_h"), gh_sc[:], start=True, stop=True)
                ht_t = wk.tile([HID, COLS], F32, name="ht_t")
                nc.scalar.activation(out=ht_t[:], in_=uh_ps[:], func=AF.Tanh,
                                     bias=b("blh"))
                zh_t = wk.tile([HID, COLS], F32, name="zh_t")
                nc.vector.tensor_mul(zh_t[:], zc_t[:], ht_t[:])
                if p == 0:
                    nc.vector.tensor_scalar_mul(henc_t[:], zh_t[:], b("probs")[:, 0:1])
                else:
                    zhp_t = wk.tile([HID, COLS], F32, name="zhp_t")
                    nc.vector.tensor_scalar_mul(zhp_t[:], zh_t[:], b("probs")[:, p:p + 1])
                    nc.vector.tensor_add(henc_t[:], henc_t[:], zhp_t[:])

            # ---- latent head ----
            h_bf = sg.tile([HID, COLS], BF16)
            nc.scalar.activation(out=h_bf[:], in_=henc_t[:], func=AF.Relu)
            mu_ps = mps.tile([LAT, COLS], F32, tag="sp", name="mu_ps")
            nc.tensor.matmul(mu_ps[:], w("w_mu"), h_bf[:], start=True, stop=True)
            mu_t = sg.tile([LAT, COLS], F32)
            nc.vector.tensor_scalar_add(mu_t[:], mu_ps[:], b("b_mu"))
            nc.scalar.dma_start(out=mu_out[:], in_=mu_t[:])
            lv_ps = mps.tile([LAT, COLS], F32, tag="sp", name="lv_ps")
            nc.tensor.matmul(lv_ps[:], w("w_lv"), h_bf[:], start=True, stop=True)
            lv_t = sg.tile([LAT, COLS], F32)
            nc.vector.tensor_scalar_add(lv_t[:], lv_ps[:], b("b_lv"))
            nc.scalar.dma_start(out=lv_out[:], in_=lv_t[:])
            std_t = wk.tile([LAT, COLS], F32, name="std_t")
            nc.scalar.activation(out=std_t[:], in_=lv_t[:], func=AF.Exp, scale=0.5)
            es_t = wk.tile([LAT, COLS], F32, name="es_t")
            nc.vector.tensor_mul(es_t[:], epsT_t[:], std_t[:])
            z_bf = sg.tile([LAT, COLS], BF16)
            nc.vector.tensor_add(z_bf[:], mu_t[:], es_t[:])
            d_ps = mps.tile([HID, COLS], F32, tag="sp", name="d_ps")
            nc.tensor.matmul(d_ps[:], w("w_dec"), z_bf[:], start=True, stop=True)
            d_bf = sg.tile([HID, COLS], BF16)
            nc.vector.tensor_scalar_add(d_bf[:], d_ps[:], b("b_dec"))

            # ---- decoder Y shard (unscaled) + AllGather ----
            ag2_in = dr.tile([COLS, 2 * D_IN], BF16)
            ag2_out = dr.tile([N, 2 * D_IN], BF16, addr_space="Shared")
            for m in range(MT):
                yd_ps = gps.tile([128, 2 * D_IN], F32, tag=f"g{2 + m % 2}", name="yd_ps")
                nc.tensor.matmul(yd_ps[:], d_bf[:, m * 128:(m + 1) * 128], w("wd_cat"),
                                 start=True, stop=True)
                ydsc = wk.tile([128, 2 * D_IN], BF16, name="ydsc")
                nc.vector.tensor_scalar_mul(ydsc[:], yd_ps[:], dis_nm[:, m:m + 1])
                nc.scalar.dma_start(out=ag2_in[m * 128:(m + 1) * 128, :], in_=ydsc[:])
            nc.gpsimd.collective_compute(
                "AllGather", mybir.AluOpType.bypass,
                ins=[ag2_in[:].opt()], outs=[ag2_out[:].opt()],
                replica_groups=[list(range(NCORES))],
            )

            # ---- decoder big matmul (two M=64 gates) + gates + recon ----
            gdz_ps = gps.tile([D_IN, COLS], F32, tag="g0", name="gdz_ps")
            gdh_ps = gps.tile([D_IN, COLS], F32, tag="g1", name="gdh_ps")
            for kb in range(KT // KB):
                yd4 = yd4_p.tile([128, KB, 2 * D_IN], BF16, name="yd4")
                nc.scalar.dma_start(
                    out=yd4[:],
                    in_=ag2_out[kb * KB * 128:(kb + 1) * KB * 128, :]
                    .rearrange("(b p) f -> p b f", p=128))
                for kk in range(KB):
                    ki = kb * KB + kk
                    nc.tensor.matmul(gdz_ps[:], yd4[:, kk, 0:D_IN], a_tile(ki),
                                     start=(ki == 0), stop=(ki == KT - 1))
                    nc.tensor.matmul(gdh_ps[:], yd4[:, kk, D_IN:2 * D_IN], a_tile(ki),
                                     start=(ki == 0), stop=(ki == KT - 1))
            gdz_sc = wk.tile([D_IN, COLS], BF16, name="gdz_sc")
            nc.vector.tensor_mul(gdz_sc[:], gdz_ps[:], dis_bc[0:D_IN, :])
            uzd_ps = mps.tile([D_IN, COLS], F32, tag="uda", name="uzd_ps")
            nc.tensor.matmul(uzd_ps[:], w("wld_z"), gdz_sc[:], start=True, stop=True)
            zcd_t = wk.tile([D_IN, COLS], F32, name="zcd_t")
            nc.scalar.activation(out=zcd_t[:], in_=uzd_ps[:], func=AF.Sigmoid,
                                 bias=b("nbldz"), scale=-1.0)
            gdh_sc = wk.tile([D_IN, COLS], BF16, name="gdh_sc")
            nc.vector.tensor_mul(gdh_sc[:], gdh_ps[:], dis_bc[0:D_IN, :])
            uhd_ps = mps.tile([D_IN, COLS], F32, tag="uda", name="uhd_ps")
            nc.tensor.matmul(uhd_ps[:], w("wld_h"), gdh_sc[:], start=True, stop=True)
            htd_t = wk.tile([D_IN, COLS], F32, name="htd_t")
            nc.scalar.activation(out=htd_t[:], in_=uhd_ps[:], func=AF.Tanh,
                                 bias=b("bldh"))
            prod_t = wk.tile([D_IN, COLS], F32, name="prod_t")
            nc.vector.tensor_mul(prod_t[:], zcd_t[:], htd_t[:])
            rec_t = wk.tile([D_IN, COLS], F32, name="rec_t")
            nc.vector.tensor_scalar_max(rec_t[:], prod_t[:], 0.0)
            nc.scalar.dma_start(out=rec_out[:], in_=rec_t[:])

    nc.compile()
    return nc


def _get_nc():
    if "nc" not in _CACHE:
        _CACHE["nc"] = _build()
    return _CACHE["nc"]


def _eps():
    if "eps" not in _CACHE:
        import jax

        with jax.default_device(jax.devices("cpu")[0]):
            e = jax.random.normal(jax.random.key(42), (N, LAT), jax.numpy.float32)
        _CACHE["eps"] = np.asarray(e)
    return _CACHE["eps"]


def _np(v):
    return np.asarray(v, dtype=np.float32)


def _pack_T(arr_rc, feat):
    # (COLS, P, feat) -> (feat, P*COLS), period-major column blocks, bf16
    a = arr_rc.transpose(1, 2, 0)  # (P, feat, COLS)
    out = np.empty((feat, P * COLS), dtype=BF)
    for p in range(P):
        out[:, p * COLS:(p + 1) * COLS] = a[p].astype(BF)
    return out


def make_in_maps(x, entity_emb, time_emb, params):
    x = _np(x)
    ee = _np(entity_emb)
    te = _np(time_emb)
    p = params
    t1, td = p["t1"], p["td"]
    eps = _eps()

    wc = np.concatenate([_np(t1["Wc_z"]), _np(t1["Wc_h"])], 1)
    wvals = {
        "w_ent": _np(p["W_ent"]), "w_time": _np(p["W_time"]),
        "wc_x": wc[:D_IN], "wc_e": wc[D_IN:D_IN + HID], "wc_t": wc[D_IN + HID:],
        "wl_z": _np(t1["Wl_z"])[:HID], "wl_h": _np(t1["Wl_h"])[:HID],
        "w_mu": _np(p["W_mu"]), "w_lv": _np(p["W_lv"]), "w_dec": _np(p["W_dec"]),
        "wd_cat": np.concatenate([_np(td["Wc_z"]), _np(td["Wc_h"])], 1),
        "wld_z": _np(td["Wl_z"])[:D_IN], "wld_h": _np(td["Wl_h"])[:D_IN],
    }
    wblob = np.zeros((128, WBLOB_COLS), dtype=BF)
    for name, (o, pp, c) in WOFF.items():
        wblob[0:pp, o:o + c] = wvals[name].astype(BF)

    att = _np(p["att1"])
    pr = np.exp(att - att.max())
    pr = (pr / pr.sum()).astype(np.float32)
    bvals = {
        "nblz": -(_np(t1["bc_z"]) @ _np(t1["Wl_z"])[:HID] + _np(t1["bl_z"])).reshape(HID, 1),
        "blh": (_np(t1["bc_h"]) @ _np(t1["Wl_h"])[:HID] + _np(t1["bl_h"])).reshape(HID, 1),
        "b_mu": _np(p["b_mu"]).reshape(LAT, 1),
        "b_lv": _np(p["b_lv"]).reshape(LAT, 1),
        "b_dec": _np(p["b_dec"]).reshape(HID, 1),
        "nbldz": -(_np(td["bc_z"]) @ _np(td["Wl_z"])[:D_IN] + _np(td["bl_z"])).reshape(D_IN, 1),
        "bldh": (_np(td["bc_h"]) @ _np(td["Wl_h"])[:D_IN] + _np(td["bl_h"])).reshape(D_IN, 1),
        "probs": np.broadcast_to(pr, (128, P)),
        "id4": np.eye(4, dtype=np.float32),
        "id32": np.eye(32, dtype=np.float32),
    }
    bblob = np.zeros((128, BBLOB_COLS), dtype=np.float32)
    for name, (o, pp, c) in BOFF.items():
        bblob[0:pp, o:o + c] = bvals[name].astype(np.float32)

    es_full = _np(p["edge_score"])
    in_maps = []
    for c in range(NCORES):
        rc = slice(c * COLS, (c + 1) * COLS)
        blk = es_full[:, rc]  # (4096, 512)
        esc_tiled = np.ascontiguousarray(
            blk.reshape(KT, 128, COLS).transpose(1, 0, 2).reshape(128, KT * COLS))
        in_maps.append({
            "wblob": wblob, "bblob": bblob, "esc": esc_tiled,
            "xTp": _pack_T(x[rc], D_IN),
            "eeTp": _pack_T(ee[rc], D_EMB),
            "teTp": _pack_T(te[rc], D_EMB),
            "epsT": np.ascontiguousarray(eps[rc].T),
        })
    return in_maps


def assemble(results):
    a_blocks, mu_blocks, lv_blocks, rec_blocks = [], [], [], []
    for c in range(NCORES):
        r = results[c]
        a_blocks.append(
            r["a_out"].reshape(128, KT, COLS).transpose(1, 0, 2).reshape(N, COLS))
        mu_blocks.append(r["mu_out"].T)
        lv_blocks.append(r["lv_out"].T)
        rec_blocks.append(r["rec_out"].T)
    A = np.concatenate(a_blocks, axis=1)
    mu = np.concatenate(mu_blocks, axis=0)
    lv = np.concatenate(lv_blocks, axis=0)
    rec = np.concatenate(rec_blocks, axis=0)
    return rec, mu, lv, A


def kernel(x, entity_emb, time_emb, num_nodes, params):
    nc = _get_nc()
    in_maps = make_in_maps(x, entity_emb, time_emb, params)
    res = run_bass_kernel_spmd(nc, in_maps, list(range(NCORES)))
    return assemble(res.results)


# revision 16
# speedup vs baseline: 1.0483x; 1.0442x over previous
"""Bass/Trainium2 kernel for nn_CausalGraphVAE (GCN message passing VAE).

Sharding: adjacency columns (= AnT output rows) split across 8 cores.
Per core: sigmoid+deg over its (4096,512) column block of edge_score
(in-place sigmoid, deferred A writes), dis=1/sqrt(deg) via broadcast
matmul, tiny dis AllGather, dis_i folded into the resident bf16 a-tiles,
X-side matmuls sharded by node rows with an early unscaled bf16
AllGather, big matmul A[:,rc]^T @ Ys per core, gates/latent local
(H0=0 kills the R gate; gate biases folded on host), second AllGather
for the decoder, outputs gathered on host. DMA traffic is split across
the two HWDGE rings: sync carries the big edge_score stream + A writes,
scalar carries everything else.
"""
import sys

if "/opt/trn_rl_repo" not in sys.path:
    sys.path.insert(0, "/opt/trn_rl_repo")

import numpy as np
import ml_dtypes

import concourse.bass as bass
import concourse.tile as tile
from concourse import bacc, mybir
from concourse.bass_utils import run_bass_kernel_spmd

NCORES = 8
N = 4096
COLS = N // NCORES          # 512 adjacency columns per core
KT = N // 128               # 32 contraction k-tiles
GSIZE = 4                   # k-tiles per DMA group in the stream
NG = KT // GSIZE            # 4 stream groups
D_IN = 64
D_EMB = 128
HID = 128
LAT = 64
P = 3
YF = P * 2 * HID            # 768 encoder Y features (z|h per period)
KB = 4                      # k-tiles per lhsT DMA batch in big matmuls
F32 = mybir.dt.float32
BF16 = mybir.dt.bfloat16
BF = ml_dtypes.bfloat16

# weight blob layout: name -> (offset, partitions, cols)
WSPEC = [
    ("w_ent", D_EMB, HID), ("w_time", D_EMB, HID),
    ("wc_x", D_IN, 2 * HID), ("wc_e", HID, 2 * HID), ("wc_t", HID, 2 * HID),
    ("wl_z", HID, HID), ("wl_h", HID, HID),
    ("w_mu", HID, LAT), ("w_lv", HID, LAT), ("w_dec", LAT, HID),
    ("wd_cat", HID, 2 * D_IN), ("wld_z", D_IN, D_IN), ("wld_h", D_IN, D_IN),
]
WOFF = {}
_o = 0
for _n, _p, _c in WSPEC:
    WOFF[_n] = (_o, _p, _c)
    _o += _c
WBLOB_COLS = _o

BSPEC = [("nblz", HID, 1), ("blh", HID, 1), ("b_mu", LAT, 1), ("b_lv", LAT, 1),
         ("b_dec", HID, 1), ("nbldz", D_IN, 1), ("bldh", D_IN, 1), ("probs", 128, P),
         ("id4", 4, 4), ("id32", 32, 32)]
BOFF = {}
_o = 0
for _n, _p, _c in BSPEC:
    BOFF[_n] = (_o, _p, _c)
    _o += _c
BBLOB_COLS = _o

_CACHE = {}


def _build():
    nc = bacc.Bacc("TRN2", debug=False, num_devices=NCORES)
    AF = mybir.ActivationFunctionType

    esc = nc.dram_tensor("esc", [128, KT * COLS], F32, kind="ExternalInput")
    xTp = nc.dram_tensor("xTp", [D_IN, P * COLS], BF16, kind="ExternalInput")
    eeTp = nc.dram_tensor("eeTp", [D_EMB, P * COLS], BF16, kind="ExternalInput")
    teTp = nc.dram_tensor("teTp", [D_EMB, P * COLS], BF16, kind="ExternalInput")
    epsT = nc.dram_tensor("epsT", [LAT, COLS], F32, kind="ExternalInput")
    wblob = nc.dram_tensor("wblob", [128, WBLOB_COLS], BF16, kind="ExternalInput")
    bblob = nc.dram_tensor("bblob", [128, BBLOB_COLS], F32, kind="ExternalInput")

    a_out = nc.dram_tensor("a_out", [128, KT * COLS], F32, kind="ExternalOutput")
    mu_out = nc.dram_tensor("mu_out", [LAT, COLS], F32, kind="ExternalOutput")
    lv_out = nc.dram_tensor("lv_out", [LAT, COLS], F32, kind="ExternalOutput")
    rec_out = nc.dram_tensor("rec_out", [D_IN, COLS], F32, kind="ExternalOutput")

    dis_dram = nc.dram_tensor("dis_dram", [1, COLS], F32)
    dis_full = nc.dram_tensor("dis_full", [KT, 128], F32, addr_space="Shared")

    with tile.TileContext(nc) as tc:
        with (
            tc.tile_pool(name="singles", bufs=1) as sg,
            tc.tile_pool(name="esc_in", bufs=8) as esc_p,
            tc.tile_pool(name="ys4", bufs=3) as ys4_p,
            tc.tile_pool(name="yd4", bufs=2) as yd4_p,
            tc.tile_pool(name="work", bufs=1) as wk,
            tc.tile_pool(name="gps", bufs=1, space="PSUM") as gps,
            tc.tile_pool(name="mps", bufs=1, space="PSUM") as mps,
            tc.tile_pool(name="dram", bufs=1, space="DRAM") as dr,
        ):
            # ---- small loads first (scalar ring, blob DMAs) ----
            wblob_t = sg.tile([128, WBLOB_COLS], BF16)
            nc.scalar.dma_start(out=wblob_t[:], in_=wblob[:])
            bblob_t = sg.tile([128, BBLOB_COLS], F32)
            nc.scalar.dma_start(out=bblob_t[:], in_=bblob[:])

            def w(name):
                o, p, c = WOFF[name]
                return wblob_t[0:p, o:o + c]

            def b(name):
                o, p, c = BOFF[name]
                return bblob_t[0:p, o:o + c]

            xT_t = sg.tile([D_IN, P * COLS], BF16)
            nc.scalar.dma_start(out=xT_t[:], in_=xTp[:])
            eeT_t = sg.tile([D_EMB, P * COLS], BF16)
            nc.scalar.dma_start(out=eeT_t[:], in_=eeTp[:])
            teT_t = sg.tile([D_EMB, P * COLS], BF16)
            nc.scalar.dma_start(out=teT_t[:], in_=teTp[:])
            epsT_t = sg.tile([LAT, COLS], F32)
            nc.scalar.dma_start(out=epsT_t[:], in_=epsT[:])
            ones_t = sg.tile([128, 1], BF16)
            nc.vector.memset(ones_t[:], 1.0)
            ones_row = sg.tile([1, 128], F32)
            nc.vector.memset(ones_row[:], 1.0)

            # absorb inter-core start skew with an early no-payload sync
            sync_sb = sg.tile([1, 16], F32)
            nc.vector.memset(sync_sb[:], 0.0)
            sync_in = dr.tile([1, 16], F32)
            sync_out = dr.tile([NCORES, 16], F32, addr_space="Shared")
            nc.gpsimd.dma_start(out=sync_in[:], in_=sync_sb[:])
            nc.gpsimd.collective_compute(
                "AllGather", mybir.AluOpType.bypass,
                ins=[sync_in[:].opt()], outs=[sync_out[:].opt()],
                replica_groups=[list(range(NCORES))],
            )

            # ---- ent/tim features first (relu on DVE to avoid ACT FIFO) ----
            ent_t = sg.tile([HID, P * COLS], BF16)
            tim_t = sg.tile([HID, P * COLS], BF16)
            for p in range(P):
                psl = slice(p * COLS, (p + 1) * COLS)
                ps1 = gps.tile([HID, COLS], F32, tag="g2", name="ent_ps")
                nc.tensor.matmul(ps1[:], w("w_ent"), eeT_t[:, psl], start=True, stop=True)
                nc.vector.tensor_scalar_max(ent_t[:, psl], ps1[:], 0.0)
                ps2 = gps.tile([HID, COLS], F32, tag="g3", name="tim_ps")
                nc.tensor.matmul(ps2[:], w("w_time"), teT_t[:, psl], start=True, stop=True)
                nc.vector.tensor_scalar_max(tim_t[:, psl], ps2[:], 0.0)

            # ---- local Y shard: drain psum to f32 SBUF early ----
            MT = COLS // 128
            y_sb = []
            for p in range(P):
                for m in range(MT):
                    msl = slice(m * 128, (m + 1) * 128)
                    psl = slice(p * COLS, (p + 1) * COLS)
                    y_ps = gps.tile([128, 2 * HID], F32, tag=f"g{m % 2}", name="y_ps")
                    nc.tensor.matmul(y_ps[:], xT_t[:, psl][:, msl], w("wc_x"),
                                     start=True, stop=False)
                    nc.tensor.matmul(y_ps[:], ent_t[:, psl][:, msl], w("wc_e"),
                                     start=False, stop=False)
                    nc.tensor.matmul(y_ps[:], tim_t[:, psl][:, msl], w("wc_t"),
                                     start=False, stop=True)
                    ysb = sg.tile([128, 2 * HID], F32, name=f"ysb{p}_{m}")
                    nc.vector.tensor_copy(out=ysb[:], in_=y_ps[:])
                    y_sb.append((p, m, ysb))

            # ---- stream: esc -> sigmoid (in place) -> bf16 cast -> deg ----
            deg_ps = mps.tile([1, COLS], F32, tag="uda", name="deg_ps")
            esc_ts = []
            a_bf = []
            W = GSIZE * COLS
            for g in range(NG):
                esc_t = esc_p.tile([128, W], F32, name="esc_t")
                ring = nc.sync if g % 2 == 0 else nc.scalar
                ring.dma_start(out=esc_t[:], in_=esc[:, g * W:(g + 1) * W])
                nc.scalar.activation(out=esc_t[:], in_=esc_t[:], func=AF.Sigmoid)
                esc_ts.append(esc_t)
                ab = sg.tile([128, W], BF16, name=f"a_bf{g}")
                nc.vector.tensor_copy(out=ab[:], in_=esc_t[:])
                a_bf.append(ab)
                for kk in range(GSIZE):
                    nc.tensor.matmul(
                        deg_ps[:], ones_t[:], ab[:, kk * COLS:(kk + 1) * COLS],
                        start=(g == 0 and kk == 0), stop=(g == NG - 1 and kk == GSIZE - 1),
                    )

            def a_tile(ki):
                return a_bf[ki // GSIZE][:, (ki % GSIZE) * COLS:(ki % GSIZE + 1) * COLS]

            # ---- dis = 1/sqrt(deg): row chain, broadcast, tiny AllGather ----
            deg_sb = sg.tile([1, COLS], F32)
            nc.vector.tensor_copy(out=deg_sb[:], in_=deg_ps[:])
            sq_row = sg.tile([1, COLS], F32)
            nc.scalar.activation(out=sq_row[:], in_=deg_sb[:], func=AF.Sqrt)
            dis_row = sg.tile([1, COLS], F32)
            rscr = sg.tile([1, COLS], F32)
            nc.vector.reciprocal_approx_accurate(out=dis_row[:], in_=sq_row[:], scratch=rscr[:])
            bc_ps = mps.tile([128, COLS], F32, tag="sp", name="bc_ps")
            nc.tensor.matmul(bc_ps[:], ones_row[:], dis_row[:], start=True, stop=True)
            dis_bc = sg.tile([128, COLS], F32)
            nc.vector.tensor_copy(out=dis_bc[:], in_=bc_ps[:])
            nc.gpsimd.dma_start(out=dis_dram[:], in_=dis_row[:])
            dis4 = sg.tile([MT, 128], F32)
            nc.gpsimd.dma_start(
                out=dis4[:], in_=dis_dram[0, :].rearrange("(m p) -> m p", m=MT))
            tp_ps = mps.tile([128, MT], F32, tag="sp", name="tp_ps")
            nc.tensor.transpose(tp_ps[:], dis4[:], b("id4"))
            dis_nm = sg.tile([128, MT], F32)
            nc.vector.tensor_copy(out=dis_nm[:], in_=tp_ps[:])

            # ---- scale Y shard rows by dis_i, ship, AllGather in two halves ----
            ag1_in = dr.tile([COLS, YF], BF16)
            ag1_outA = dr.tile([N // 2, YF], BF16, addr_space="Shared")
            ag1_outB = dr.tile([N // 2, YF], BF16, addr_space="Shared")
            for p, m, ysb in y_sb:
                ysc = wk.tile([128, 2 * HID], BF16, name="ysc")
                nc.vector.tensor_scalar_mul(ysc[:], ysb[:], dis_nm[:, m:m + 1])
                nc.scalar.dma_start(
                    out=ag1_in[m * 128:(m + 1) * 128, p * 2 * HID:(p + 1) * 2 * HID],
                    in_=ysc[:])
            nc.gpsimd.collective_compute(
                "AllGather", mybir.AluOpType.bypass,
                ins=[ag1_in[0:COLS // 2, :].opt()], outs=[ag1_outA[:].opt()],
                replica_groups=[list(range(NCORES))],
            )
            nc.gpsimd.collective_compute(
                "AllGather", mybir.AluOpType.bypass,
                ins=[ag1_in[COLS // 2:COLS, :].opt()], outs=[ag1_outB[:].opt()],
                replica_groups=[list(range(NCORES))],
            )

            # ---- deferred A output writes (data-dep gated on deg) ----
            gate_t = sg.tile([1, 1], F32)
            nc.vector.tensor_scalar(gate_t[:], deg_sb[0:1, 0:1], 0.0, 1.0,
                                    mybir.AluOpType.mult, mybir.AluOpType.add)
            for g in range(NG):
                nc.vector.tensor_scalar_mul(esc_ts[g][0:1, 0:1], esc_ts[g][0:1, 0:1],
                                            gate_t[0:1, 0:1])
                nc.sync.dma_start(out=a_out[:, g * W:(g + 1) * W], in_=esc_ts[g][:])

            # ---- encoder big matmul ----
            g_ps = [gps.tile([128, COLS], F32, tag=f"g{ft}", name=f"g_ps{ft}")
                    for ft in range(6)]
            ki_order = []
            for half in (0, 1):
                for c in range(NCORES):
                    for t in (0, 1):
                        ki_order.append((c * 4 + half * 2 + t, half))
            first_ki = ki_order[0][0]
            last_ki = ki_order[-1][0]
            for idx in range(0, len(ki_order), KB):
                batch = ki_order[idx:idx + KB]
                half = batch[0][1]
                buf = ag1_outA if half == 0 else ag1_outB
                row0 = (idx % 16)  # row offset within this half's buffer
                ys4 = ys4_p.tile([128, KB, YF], BF16, name="ys4")
                nc.scalar.dma_start(
                    out=ys4[:],
                    in_=buf[row0 * 128:(row0 + KB) * 128, :]
                    .rearrange("(b p) f -> p b f", p=128))
                for j, (ki, _) in enumerate(batch):
                    for ft in range(6):
                        nc.tensor.matmul(
                            g_ps[ft][:], ys4[:, j, ft * 128:(ft + 1) * 128], a_tile(ki),
                            start=(ki == first_ki), stop=(ki == last_ki))

            # ---- encoder gates + Henc ----
            henc_t = sg.tile([HID, COLS], F32)
            for p in range(P):
                gz_sc = wk.tile([128, COLS], BF16, name="gz_sc")
                nc.vector.tensor_mul(gz_sc[:], g_ps[2 * p][:], dis_bc[:])
                u_ps = mps.tile([128, COLS], F32, tag="uda", name="uz_ps")
                nc.tensor.matmul(u_ps[:], w("wl_z"), gz_sc[:], start=True, stop=True)
                zc_t = wk.tile([HID, COLS], F32, name="zc_t")
                nc.scalar.activation(out=zc_t[:], in_=u_ps[:], func=AF.Sigmoid,
                                     bias=b("nblz"), scale=-1.0)
                gh_sc = wk.tile([128, COLS], BF16, name="gh_sc")
                nc.vector.tensor_mul(gh_sc[:], g_ps[2 * p + 1][:], dis_bc[:])
                uh_ps = mps.tile([128, COLS], F32, tag="uda", name="uh_ps")
                nc.tensor.matmul(uh_ps[:], w("wl_h"), gh_sc[:], start=True, stop=True)
                ht_t = wk.tile([HID, COLS], F32, name="ht_t")
                nc.scalar.activation(out=ht_t[:], in_=uh_ps[:], func=AF.Tanh,
                                     bias=b("blh"))
                zh_t = wk.tile([HID, COLS], F32, name="zh_t")
                nc.vector.tensor_mul(zh_t[:], zc_t[:], ht_t[:])
                if p == 0:
                    nc.vector.tensor_scalar_mul(henc_t[:], zh_t[:], b("probs")[:, 0:1])
                else:
                    zhp_t = wk.tile([HID, COLS], F32, name="zhp_t")
                    nc.vector.tensor_scalar_mul(zhp_t[:], zh_t[:], b("probs")[:, p:p + 1])
                    nc.vector.tensor_add(henc_t[:], henc_t[:], zhp_t[:])

            # ---- latent head ----
            h_bf = sg.tile([HID, COLS], BF16)
            nc.scalar.activation(out=h_bf[:], in_=henc_t[:], func=AF.Relu)
            mu_ps = mps.tile([LAT, COLS], F32, tag="sp", name="mu_ps")
            nc.tensor.matmul(mu_ps[:], w("w_mu"), h_bf[:], start=True, stop=True)
            mu_t = sg.tile([LAT, COLS], F32)
            nc.vector.tensor_scalar_add(mu_t[:], mu_ps[:], b("b_mu"))
            nc.scalar.dma_start(out=mu_out[:], in_=mu_t[:])
            lv_ps = mps.tile([LAT, COLS], F32, tag="sp", name="lv_ps")
            nc.tensor.matmul(lv_ps[:], w("w_lv"), h_bf[:], start=True, stop=True)
            lv_t = sg.tile([LAT, COLS], F32)
            nc.vector.tensor_scalar_add(lv_t[:], lv_ps[:], b("b_lv"))
            nc.scalar.dma_start(out=lv_out[:], in_=lv_t[:])
            std_t = wk.tile([LAT, COLS], F32, name="std_t")
            nc.scalar.activation(out=std_t[:], in_=lv_t[:], func=AF.Exp, scale=0.5)
            es_t = wk.tile([LAT, COLS], F32, name="es_t")
            nc.vector.tensor_mul(es_t[:], epsT_t[:], std_t[:])
            z_bf = sg.tile([LAT, COLS], BF16)
            nc.vector.tensor_add(z_bf[:], mu_t[:], es_t[:])
            d_ps = mps.tile([HID, COLS], F32, tag="sp", name="d_ps")
            nc.tensor.matmul(d_ps[:], w("w_dec"), z_bf[:], start=True, stop=True)
            d_bf = sg.tile([HID, COLS], BF16)
            nc.vector.tensor_scalar_add(d_bf[:], d_ps[:], b("b_dec"))

            # ---- decoder Y shard (unscaled) + AllGather ----
            ag2_in = dr.tile([COLS, 2 * D_IN], BF16)
            ag2_out = dr.tile([N, 2 * D_IN], BF16, addr_space="Shared")
            for m in range(MT):
                yd_ps = gps.tile([128, 2 * D_IN], F32, tag=f"g{2 + m % 2}", name="yd_ps")
                nc.tensor.matmul(yd_ps[:], d_bf[:, m * 128:(m + 1) * 128], w("wd_cat"),
                                 start=True, stop=True)
                ydsc = wk.tile([128, 2 * D_IN], BF16, name="ydsc")
                nc.vector.tensor_scalar_mul(ydsc[:], yd_ps[:], dis_nm[:, m:m + 1])
                nc.scalar.dma_start(out=ag2_in[m * 128:(m + 1) * 128, :], in_=ydsc[:])
            nc.gpsimd.collective_compute(
                "AllGather", mybir.AluOpType.bypass,
                ins=[ag2_in[:].opt()], outs=[ag2_out[:].opt()],
                replica_groups=[list(range(NCORES))],
            )

            # ---- decoder big matmul (two M=64 gates) + gates + recon ----
            gdz_ps = gps.tile([D_IN, COLS], F32, tag="g0", name="gdz_ps")
            gdh_ps = gps.tile([D_IN, COLS], F32, tag="g1", name="gdh_ps")
            for kb in range(KT // KB):
                yd4 = yd4_p.tile([128, KB, 2 * D_IN], BF16, name="yd4")
                nc.scalar.dma_start(
                    out=yd4[:],
                    in_=ag2_out[kb * KB * 128:(kb + 1) * KB * 128, :]
                    .rearrange("(b p) f -> p b f", p=128))
                for kk in range(KB):
                    ki = kb * KB + kk
                    nc.tensor.matmul(gdz_ps[:], yd4[:, kk, 0:D_IN], a_tile(ki),
                                     start=(ki == 0), stop=(ki == KT - 1))
                    nc.tensor.matmul(gdh_ps[:], yd4[:, kk, D_IN:2 * D_IN], a_tile(ki),
                                     start=(ki == 0), stop=(ki == KT - 1))
            gdz_sc = wk.tile([D_IN, COLS], BF16, name="gdz_sc")
            nc.vector.tensor_mul(gdz_sc[:], gdz_ps[:], dis_bc[0:D_IN, :])
            uzd_ps = mps.tile([D_IN, COLS], F32, tag="uda", name="uzd_ps")
            nc.tensor.matmul(uzd_ps[:], w("wld_z"), gdz_sc[:], start=True, stop=True)
            zcd_t = wk.tile([D_IN, COLS], F32, name="zcd_t")
            nc.scalar.activation(out=zcd_t[:], in_=uzd_ps[:], func=AF.Sigmoid,
                                 bias=b("nbldz"), scale=-1.0)
            gdh_sc = wk.tile([D_IN, COLS], BF16, name="gdh_sc")
            nc.vector.tensor_mul(gdh_sc[:], gdh_ps[:], dis_bc[0:D_IN, :])
            uhd_ps = mps.tile([D_IN, COLS], F32, tag="uda", name="uhd_ps")
            nc.tensor.matmul(uhd_ps[:], w("wld_h"), gdh_sc[:], start=True, stop=True)
            htd_t = wk.tile([D_IN, COLS], F32, name="htd_t")
            nc.scalar.activation(out=htd_t[:], in_=uhd_ps[:], func=AF.Tanh,
                                 bias=b("bldh"))
            prod_t = wk.tile([D_IN, COLS], F32, name="prod_t")
            nc.vector.tensor_mul(prod_t[:], zcd_t[:], htd_t[:])
            rec_t = wk.tile([D_IN, COLS], F32, name="rec_t")
            nc.vector.tensor_scalar_max(rec_t[:], prod_t[:], 0.0)
            nc.scalar.dma_start(out=rec_out[:], in_=rec_t[:])

    nc.compile()
    return nc


def _get_nc():
    if "nc" not in _CACHE:
        _CACHE["nc"] = _build()
    return _CACHE["nc"]


def _eps():
    if "eps" not in _CACHE:
        import jax

        with jax.default_device(jax.devices("cpu")[0]):
            e = jax.random.normal(jax.random.key(42), (N, LAT), jax.numpy.float32)
        _CACHE["eps"] = np.asarray(e)
    return _CACHE["eps"]


def _np(v):
    return np.asarray(v, dtype=np.float32)


def _pack_T(arr_rc, feat):
    # (COLS, P, feat) -> (feat, P*COLS), period-major column blocks, bf16
    a = arr_rc.transpose(1, 2, 0)  # (P, feat, COLS)
    out = np.empty((feat, P * COLS), dtype=BF)
    for p in range(P):
        out[:, p * COLS:(p + 1) * COLS] = a[p].astype(BF)
    return out


def make_in_maps(x, entity_emb, time_emb, params):
    x = _np(x)
    ee = _np(entity_emb)
    te = _np(time_emb)
    p = params
    t1, td = p["t1"], p["td"]
    eps = _eps()

    wc = np.concatenate([_np(t1["Wc_z"]), _np(t1["Wc_h"])], 1)
    wvals = {
        "w_ent": _np(p["W_ent"]), "w_time": _np(p["W_time"]),
        "wc_x": wc[:D_IN], "wc_e": wc[D_IN:D_IN + HID], "wc_t": wc[D_IN + HID:],
        "wl_z": _np(t1["Wl_z"])[:HID], "wl_h": _np(t1["Wl_h"])[:HID],
        "w_mu": _np(p["W_mu"]), "w_lv": _np(p["W_lv"]), "w_dec": _np(p["W_dec"]),
        "wd_cat": np.concatenate([_np(td["Wc_z"]), _np(td["Wc_h"])], 1),
        "wld_z": _np(td["Wl_z"])[:D_IN], "wld_h": _np(td["Wl_h"])[:D_IN],
    }
    wblob = np.zeros((128, WBLOB_COLS), dtype=BF)
    for name, (o, pp, c) in WOFF.items():
        wblob[0:pp, o:o + c] = wvals[name].astype(BF)

    att = _np(p["att1"])
    pr = np.exp(att - att.max())
    pr = (pr / pr.sum()).astype(np.float32)
    bvals = {
        "nblz": -(_np(t1["bc_z"]) @ _np(t1["Wl_z"])[:HID] + _np(t1["bl_z"])).reshape(HID, 1),
        "blh": (_np(t1["bc_h"]) @ _np(t1["Wl_h"])[:HID] + _np(t1["bl_h"])).reshape(HID, 1),
        "b_mu": _np(p["b_mu"]).reshape(LAT, 1),
        "b_lv": _np(p["b_lv"]).reshape(LAT, 1),
        "b_dec": _np(p["b_dec"]).reshape(HID, 1),
        "nbldz": -(_np(td["bc_z"]) @ _np(td["Wl_z"])[:D_IN] + _np(td["bl_z"])).reshape(D_IN, 1),
        "bldh": (_np(td["bc_h"]) @ _np(td["Wl_h"])[:D_IN] + _np(td["bl_h"])).reshape(D_IN, 1),
        "probs": np.broadcast_to(pr, (128, P)),
        "id4": np.eye(4, dtype=np.float32),
        "id32": np.eye(32, dtype=np.float32),
    }
    bblob = np.zeros((128, BBLOB_COLS), dtype=np.float32)
    for name, (o, pp, c) in BOFF.items():
        bblob[0:pp, o:o + c] = bvals[name].astype(np.float32)

    es_full = _np(p["edge_score"])
    in_maps = []
    for c in range(NCORES):
        rc = slice(c * COLS, (c + 1) * COLS)
        blk = es_full[:, rc]  # (4096, 512)
        esc_tiled = np.ascontiguousarray(
            blk.reshape(KT, 128, COLS).transpose(1, 0, 2).reshape(128, KT * COLS))
        in_maps.append({
            "wblob": wblob, "bblob": bblob, "esc": esc_tiled,
            "xTp": _pack_T(x[rc], D_IN),
            "eeTp": _pack_T(ee[rc], D_EMB),
            "teTp": _pack_T(te[rc], D_EMB),
            "epsT": np.ascontiguousarray(eps[rc].T),
        })
    return in_maps


def assemble(results):
    a_blocks, mu_blocks, lv_blocks, rec_blocks = [], [], [], []
    for c in range(NCORES):
        r = results[c]
        a_blocks.append(
            r["a_out"].reshape(128, KT, COLS).transpose(1, 0, 2).reshape(N, COLS))
        mu_blocks.append(r["mu_out"].T)
        lv_blocks.append(r["lv_out"].T)
        rec_blocks.append(r["rec_out"].T)
    A = np.concatenate(a_blocks, axis=1)
    mu = np.concatenate(mu_blocks, axis=0)
    lv = np.concatenate(lv_blocks, axis=0)
    rec = np.concatenate(rec_blocks, axis=0)
    return rec, mu, lv, A


def kernel(x, entity_emb, time_emb, num_nodes, params):
    nc = _get_nc()
    in_maps = make_in_maps(x, entity_emb, time_emb, params)
    res = run_bass_kernel_spmd(nc, in_maps, list(range(NCORES)))
    return assemble(res.results)


# revision 18
# speedup vs baseline: 1.1104x; 1.0592x over previous
"""Bass/Trainium2 kernel for nn_CausalGraphVAE (GCN message passing VAE).

Sharding: adjacency columns (= AnT output rows) split across 8 cores.
Per core: sigmoid+deg over its (4096,512) column block of edge_score
(in-place sigmoid, deferred A writes), dis=1/sqrt(deg) via broadcast
matmul, tiny dis AllGather, dis_i folded into the resident bf16 a-tiles,
X-side matmuls sharded by node rows with an early unscaled bf16
AllGather, big matmul A[:,rc]^T @ Ys per core, gates/latent local
(H0=0 kills the R gate; gate biases folded on host), second AllGather
for the decoder, outputs gathered on host. DMA traffic is split across
the two HWDGE rings: sync carries the big edge_score stream + A writes,
scalar carries everything else.
"""
import sys

if "/opt/trn_rl_repo" not in sys.path:
    sys.path.insert(0, "/opt/trn_rl_repo")

import numpy as np
import ml_dtypes

import concourse.bass as bass
import concourse.tile as tile
from concourse import bacc, mybir
from concourse.bass_utils import run_bass_kernel_spmd

NCORES = 8
N = 4096
COLS = N // NCORES          # 512 adjacency columns per core
KT = N // 128               # 32 contraction k-tiles
GSIZE = 8                   # k-tiles per DMA group in the stream
NG = KT // GSIZE            # 4 stream groups
D_IN = 64
D_EMB = 128
HID = 128
LAT = 64
P = 3
YF = P * 2 * HID            # 768 encoder Y features (z|h per period)
KB = 4                      # k-tiles per lhsT DMA batch in big matmuls
F32 = mybir.dt.float32
BF16 = mybir.dt.bfloat16
BF = ml_dtypes.bfloat16

# weight blob layout: name -> (offset, partitions, cols)
WSPEC = [
    ("w_ent", D_EMB, HID), ("w_time", D_EMB, HID),
    ("wc_x", D_IN, 2 * HID), ("wc_e", HID, 2 * HID), ("wc_t", HID, 2 * HID),
    ("wl_z", HID, HID), ("wl_h", HID, HID),
    ("w_mu", HID, LAT), ("w_lv", HID, LAT), ("w_dec", LAT, HID),
    ("wd_cat", HID, 2 * D_IN), ("wld_z", D_IN, D_IN), ("wld_h", D_IN, D_IN),
]
WOFF = {}
_o = 0
for _n, _p, _c in WSPEC:
    WOFF[_n] = (_o, _p, _c)
    _o += _c
WBLOB_COLS = _o

BSPEC = [("nblz", HID, 1), ("blh", HID, 1), ("b_mu", LAT, 1), ("b_lv", LAT, 1),
         ("b_dec", HID, 1), ("nbldz", D_IN, 1), ("bldh", D_IN, 1), ("probs", 128, P),
         ("id4", 4, 4), ("id32", 32, 32)]
BOFF = {}
_o = 0
for _n, _p, _c in BSPEC:
    BOFF[_n] = (_o, _p, _c)
    _o += _c
BBLOB_COLS = _o

_CACHE = {}


def _build():
    nc = bacc.Bacc("TRN2", debug=False, num_devices=NCORES)
    AF = mybir.ActivationFunctionType

    esc = nc.dram_tensor("esc", [128, KT * COLS], F32, kind="ExternalInput")
    xTp = nc.dram_tensor("xTp", [D_IN, P * COLS], BF16, kind="ExternalInput")
    eeTp = nc.dram_tensor("eeTp", [D_EMB, P * COLS], BF16, kind="ExternalInput")
    teTp = nc.dram_tensor("teTp", [D_EMB, P * COLS], BF16, kind="ExternalInput")
    epsT = nc.dram_tensor("epsT", [LAT, COLS], F32, kind="ExternalInput")
    wblob = nc.dram_tensor("wblob", [128, WBLOB_COLS], BF16, kind="ExternalInput")
    bblob = nc.dram_tensor("bblob", [128, BBLOB_COLS], F32, kind="ExternalInput")

    a_out = nc.dram_tensor("a_out", [128, KT * COLS], F32, kind="ExternalOutput")
    mu_out = nc.dram_tensor("mu_out", [LAT, COLS], F32, kind="ExternalOutput")
    lv_out = nc.dram_tensor("lv_out", [LAT, COLS], F32, kind="ExternalOutput")
    rec_out = nc.dram_tensor("rec_out", [D_IN, COLS], F32, kind="ExternalOutput")

    dis_dram = nc.dram_tensor("dis_dram", [1, COLS], F32)
    dis_full = nc.dram_tensor("dis_full", [KT, 128], F32, addr_space="Shared")

    with tile.TileContext(nc) as tc:
        with (
            tc.tile_pool(name="singles", bufs=1) as sg,
            tc.tile_pool(name="esc_in", bufs=4) as esc_p,
            tc.tile_pool(name="ys4", bufs=3) as ys4_p,
            tc.tile_pool(name="yd4", bufs=2) as yd4_p,
            tc.tile_pool(name="work", bufs=1) as wk,
            tc.tile_pool(name="gps", bufs=1, space="PSUM") as gps,
            tc.tile_pool(name="mps", bufs=1, space="PSUM") as mps,
            tc.tile_pool(name="dram", bufs=1, space="DRAM") as dr,
        ):
            # ---- small loads first (scalar ring, blob DMAs) ----
            wblob_t = sg.tile([128, WBLOB_COLS], BF16)
            nc.scalar.dma_start(out=wblob_t[:], in_=wblob[:])
            bblob_t = sg.tile([128, BBLOB_COLS], F32)
            nc.scalar.dma_start(out=bblob_t[:], in_=bblob[:])

            def w(name):
                o, p, c = WOFF[name]
                return wblob_t[0:p, o:o + c]

            def b(name):
                o, p, c = BOFF[name]
                return bblob_t[0:p, o:o + c]

            xT_t = sg.tile([D_IN, P * COLS], BF16)
            nc.scalar.dma_start(out=xT_t[:], in_=xTp[:])
            eeT_t = sg.tile([D_EMB, P * COLS], BF16)
            nc.scalar.dma_start(out=eeT_t[:], in_=eeTp[:])
            teT_t = sg.tile([D_EMB, P * COLS], BF16)
            nc.scalar.dma_start(out=teT_t[:], in_=teTp[:])
            epsT_t = sg.tile([LAT, COLS], F32)
            nc.scalar.dma_start(out=epsT_t[:], in_=epsT[:])
            ones_t = sg.tile([128, 1], BF16)
            nc.vector.memset(ones_t[:], 1.0)
            ones_row = sg.tile([1, 128], F32)
            nc.vector.memset(ones_row[:], 1.0)

            # absorb inter-core start skew with an early no-payload sync
            sync_sb = sg.tile([1, 16], F32)
            nc.vector.memset(sync_sb[:], 0.0)
            sync_in = dr.tile([1, 16], F32)
            sync_out = dr.tile([NCORES, 16], F32, addr_space="Shared")
            nc.gpsimd.dma_start(out=sync_in[:], in_=sync_sb[:])
            nc.gpsimd.collective_compute(
                "AllGather", mybir.AluOpType.bypass,
                ins=[sync_in[:].opt()], outs=[sync_out[:].opt()],
                replica_groups=[list(range(NCORES))],
            )

            # ---- ent/tim features first (relu on DVE to avoid ACT FIFO) ----
            ent_t = sg.tile([HID, P * COLS], BF16)
            tim_t = sg.tile([HID, P * COLS], BF16)
            for p in range(P):
                psl = slice(p * COLS, (p + 1) * COLS)
                ps1 = gps.tile([HID, COLS], F32, tag="g2", name="ent_ps")
                nc.tensor.matmul(ps1[:], w("w_ent"), eeT_t[:, psl], start=True, stop=True)
                nc.vector.tensor_scalar_max(ent_t[:, psl], ps1[:], 0.0)
                ps2 = gps.tile([HID, COLS], F32, tag="g3", name="tim_ps")
                nc.tensor.matmul(ps2[:], w("w_time"), teT_t[:, psl], start=True, stop=True)
                nc.vector.tensor_scalar_max(tim_t[:, psl], ps2[:], 0.0)

            # ---- local Y shard: drain psum to f32 SBUF early ----
            MT = COLS // 128
            y_sb = []
            for p in range(P):
                for m in range(MT):
                    msl = slice(m * 128, (m + 1) * 128)
                    psl = slice(p * COLS, (p + 1) * COLS)
                    y_ps = gps.tile([128, 2 * HID], F32, tag=f"g{m % 2}", name="y_ps")
                    nc.tensor.matmul(y_ps[:], xT_t[:, psl][:, msl], w("wc_x"),
                                     start=True, stop=False)
                    nc.tensor.matmul(y_ps[:], ent_t[:, psl][:, msl], w("wc_e"),
                                     start=False, stop=False)
                    nc.tensor.matmul(y_ps[:], tim_t[:, psl][:, msl], w("wc_t"),
                                     start=False, stop=True)
                    ysb = sg.tile([128, 2 * HID], F32, name=f"ysb{p}_{m}")
                    nc.vector.tensor_copy(out=ysb[:], in_=y_ps[:])
                    y_sb.append((p, m, ysb))

            # ---- stream: esc -> sigmoid (in place) -> bf16 cast -> deg ----
            deg_ps = mps.tile([1, COLS], F32, tag="uda", name="deg_ps")
            esc_ts = []
            a_bf = []
            W = GSIZE * COLS
            for g in range(NG):
                esc_t = esc_p.tile([128, W], F32, name="esc_t")
                ring = nc.sync if g % 2 == 0 else nc.scalar
                ring.dma_start(out=esc_t[:], in_=esc[:, g * W:(g + 1) * W])
                nc.scalar.activation(out=esc_t[:], in_=esc_t[:], func=AF.Sigmoid)
                esc_ts.append(esc_t)
                ab = sg.tile([128, W], BF16, name=f"a_bf{g}")
                nc.vector.tensor_copy(out=ab[:], in_=esc_t[:])
                a_bf.append(ab)
                for kk in range(GSIZE):
                    nc.tensor.matmul(
                        deg_ps[:], ones_t[:], ab[:, kk * COLS:(kk + 1) * COLS],
                        start=(g == 0 and kk == 0), stop=(g == NG - 1 and kk == GSIZE - 1),
                    )

            def a_tile(ki):
                return a_bf[ki // GSIZE][:, (ki % GSIZE) * COLS:(ki % GSIZE + 1) * COLS]

            # ---- dis = 1/sqrt(deg): row chain, broadcast, tiny AllGather ----
            deg_sb = sg.tile([1, COLS], F32)
            nc.vector.tensor_copy(out=deg_sb[:], in_=deg_ps[:])
            sq_row = sg.tile([1, COLS], F32)
            nc.scalar.activation(out=sq_row[:], in_=deg_sb[:], func=AF.Sqrt)
            dis_row = sg.tile([1, COLS], F32)
            rscr = sg.tile([1, COLS], F32)
            nc.vector.reciprocal_approx_accurate(out=dis_row[:], in_=sq_row[:], scratch=rscr[:])
            bc_ps = mps.tile([128, COLS], F32, tag="sp", name="bc_ps")
            nc.tensor.matmul(bc_ps[:], ones_row[:], dis_row[:], start=True, stop=True)
            dis_bc = sg.tile([128, COLS], F32)
            nc.vector.tensor_copy(out=dis_bc[:], in_=bc_ps[:])
            nc.gpsimd.dma_start(out=dis_dram[:], in_=dis_row[:])
            dis4 = sg.tile([MT, 128], F32)
            nc.gpsimd.dma_start(
                out=dis4[:], in_=dis_dram[0, :].rearrange("(m p) -> m p", m=MT))
            tp_ps = mps.tile([128, MT], F32, tag="sp", name="tp_ps")
            nc.tensor.transpose(tp_ps[:], dis4[:], b("id4"))
            dis_nm = sg.tile([128, MT], F32)
            nc.vector.tensor_copy(out=dis_nm[:], in_=tp_ps[:])

            # ---- scale Y shard rows by dis_i, ship, AllGather in two halves ----
            ag1_in = dr.tile([COLS, YF], BF16)
            ag1_outA = dr.tile([N // 2, YF], BF16, addr_space="Shared")
            ag1_outB = dr.tile([N // 2, YF], BF16, addr_space="Shared")
            for p, m, ysb in y_sb:
                ysc = wk.tile([128, 2 * HID], BF16, name="ysc")
                nc.vector.tensor_scalar_mul(ysc[:], ysb[:], dis_nm[:, m:m + 1])
                nc.scalar.dma_start(
                    out=ag1_in[m * 128:(m + 1) * 128, p * 2 * HID:(p + 1) * 2 * HID],
                    in_=ysc[:])
            nc.gpsimd.collective_compute(
                "AllGather", mybir.AluOpType.bypass,
                ins=[ag1_in[0:COLS // 2, :].opt()], outs=[ag1_outA[:].opt()],
                replica_groups=[list(range(NCORES))],
            )
            nc.gpsimd.collective_compute(
                "AllGather", mybir.AluOpType.bypass,
                ins=[ag1_in[COLS // 2:COLS, :].opt()], outs=[ag1_outB[:].opt()],
                replica_groups=[list(range(NCORES))],
            )

            # ---- deferred A output writes (data-dep gated on deg) ----
            gate_t = sg.tile([1, 1], F32)
            nc.vector.tensor_scalar(gate_t[:], deg_sb[0:1, 0:1], 0.0, 1.0,
                                    mybir.AluOpType.mult, mybir.AluOpType.add)
            for g in range(NG):
                nc.vector.tensor_scalar_mul(esc_ts[g][0:1, 0:1], esc_ts[g][0:1, 0:1],
                                            gate_t[0:1, 0:1])
                nc.sync.dma_start(out=a_out[:, g * W:(g + 1) * W], in_=esc_ts[g][:])

            # ---- encoder big matmul ----
            g_ps = [gps.tile([128, COLS], F32, tag=f"g{ft}", name=f"g_ps{ft}")
                    for ft in range(6)]
            ki_order = []
            for half in (0, 1):
                for c in range(NCORES):
                    for t in (0, 1):
                        ki_order.append((c * 4 + half * 2 + t, half))
            first_ki = ki_order[0][0]
            last_ki = ki_order[-1][0]
            for idx in range(0, len(ki_order), KB):
                batch = ki_order[idx:idx + KB]
                half = batch[0][1]
                buf = ag1_outA if half == 0 else ag1_outB
                row0 = (idx % 16)  # row offset within this half's buffer
                ys4 = ys4_p.tile([128, KB, YF], BF16, name="ys4")
                nc.scalar.dma_start(
                    out=ys4[:],
                    in_=buf[row0 * 128:(row0 + KB) * 128, :]
                    .rearrange("(b p) f -> p b f", p=128))
                for j, (ki, _) in enumerate(batch):
                    for ft in range(6):
                        nc.tensor.matmul(
                            g_ps[ft][:], ys4[:, j, ft * 128:(ft + 1) * 128], a_tile(ki),
                            start=(ki == first_ki), stop=(ki == last_ki))

            # ---- encoder gates + Henc ----
            henc_t = sg.tile([HID, COLS], F32)
            for p in range(P):
                gz_sc = wk.tile([128, COLS], BF16, name="gz_sc", bufs=2)
                nc.vector.tensor_mul(gz_sc[:], g_ps[2 * p][:], dis_bc[:])
                u_ps = mps.tile([128, COLS], F32, tag="uda", name="uz_ps")
                nc.tensor.matmul(u_ps[:], w("wl_z"), gz_sc[:], start=True, stop=True)
                zc_t = wk.tile([HID, COLS], F32, name="zc_t", bufs=2)
                nc.scalar.activation(out=zc_t[:], in_=u_ps[:], func=AF.Sigmoid,
                                     bias=b("nblz"), scale=-1.0)
                gh_sc = wk.tile([128, COLS], BF16, name="gh_sc", bufs=2)
                nc.vector.tensor_mul(gh_sc[:], g_ps[2 * p + 1][:], dis_bc[:])
                uh_ps = mps.tile([128, COLS], F32, tag="sp", name="uh_ps")
                nc.tensor.matmul(uh_ps[:], w("wl_h"), gh_sc[:], start=True, stop=True)
                ht_t = wk.tile([HID, COLS], F32, name="ht_t", bufs=2)
                nc.scalar.activation(out=ht_t[:], in_=uh_ps[:], func=AF.Tanh,
                                     bias=b("blh"))
                zh_t = wk.tile([HID, COLS], F32, name="zh_t")
                nc.vector.tensor_mul(zh_t[:], zc_t[:], ht_t[:])
                if p == 0:
                    nc.vector.tensor_scalar_mul(henc_t[:], zh_t[:], b("probs")[:, 0:1])
                else:
                    zhp_t = wk.tile([HID, COLS], F32, name="zhp_t")
                    nc.vector.tensor_scalar_mul(zhp_t[:], zh_t[:], b("probs")[:, p:p + 1])
                    nc.vector.tensor_add(henc_t[:], henc_t[:], zhp_t[:])

            # ---- latent head ----
            h_bf = sg.tile([HID, COLS], BF16)
            nc.scalar.activation(out=h_bf[:], in_=henc_t[:], func=AF.Relu)
            mu_ps = mps.tile([LAT, COLS], F32, tag="sp", name="mu_ps")
            nc.tensor.matmul(mu_ps[:], w("w_mu"), h_bf[:], start=True, stop=True)
            mu_t = sg.tile([LAT, COLS], F32)
            nc.vector.tensor_scalar_add(mu_t[:], mu_ps[:], b("b_mu"))
            nc.scalar.dma_start(out=mu_out[:], in_=mu_t[:])
            lv_ps = mps.tile([LAT, COLS], F32, tag="sp", name="lv_ps")
            nc.tensor.matmul(lv_ps[:], w("w_lv"), h_bf[:], start=True, stop=True)
            lv_t = sg.tile([LAT, COLS], F32)
            nc.vector.tensor_scalar_add(lv_t[:], lv_ps[:], b("b_lv"))
            nc.scalar.dma_start(out=lv_out[:], in_=lv_t[:])
            std_t = wk.tile([LAT, COLS], F32, name="std_t")
            nc.scalar.activation(out=std_t[:], in_=lv_t[:], func=AF.Exp, scale=0.5)
            es_t = wk.tile([LAT, COLS], F32, name="es_t")
            nc.vector.tensor_mul(es_t[:], epsT_t[:], std_t[:])
            z_bf = sg.tile([LAT, COLS], BF16)
            nc.vector.tensor_add(z_bf[:], mu_t[:], es_t[:])
            d_ps = mps.tile([HID, COLS], F32, tag="sp", name="d_ps")
            nc.tensor.matmul(d_ps[:], w("w_dec"), z_bf[:], start=True, stop=True)
            d_bf = sg.tile([HID, COLS], BF16)
            nc.vector.tensor_scalar_add(d_bf[:], d_ps[:], b("b_dec"))

            # ---- decoder Y shard (unscaled) + AllGather ----
            ag2_in = dr.tile([COLS, 2 * D_IN], BF16)
            ag2_out = dr.tile([N, 2 * D_IN], BF16, addr_space="Shared")
            for m in range(MT):
                yd_ps = gps.tile([128, 2 * D_IN], F32, tag=f"g{2 + m % 2}", name="yd_ps")
                nc.tensor.matmul(yd_ps[:], d_bf[:, m * 128:(m + 1) * 128], w("wd_cat"),
                                 start=True, stop=True)
                ydsc = wk.tile([128, 2 * D_IN], BF16, name="ydsc")
                nc.vector.tensor_scalar_mul(ydsc[:], yd_ps[:], dis_nm[:, m:m + 1])
                nc.scalar.dma_start(out=ag2_in[m * 128:(m + 1) * 128, :], in_=ydsc[:])
            nc.gpsimd.collective_compute(
                "AllGather", mybir.AluOpType.bypass,
                ins=[ag2_in[:].opt()], outs=[ag2_out[:].opt()],
                replica_groups=[list(range(NCORES))],
            )

            # ---- decoder big matmul (two M=64 gates) + gates + recon ----
            gdz_ps = gps.tile([D_IN, COLS], F32, tag="g0", name="gdz_ps")
            gdh_ps = gps.tile([D_IN, COLS], F32, tag="g1", name="gdh_ps")
            for kb in range(KT // KB):
                yd4 = yd4_p.tile([128, KB, 2 * D_IN], BF16, name="yd4")
                nc.scalar.dma_start(
                    out=yd4[:],
                    in_=ag2_out[kb * KB * 128:(kb + 1) * KB * 128, :]
                    .rearrange("(b p) f -> p b f", p=128))
                for kk in range(KB):
                    ki = kb * KB + kk
                    nc.tensor.matmul(gdz_ps[:], yd4[:, kk, 0:D_IN], a_tile(ki),
                                     start=(ki == 0), stop=(ki == KT - 1))
                    nc.tensor.matmul(gdh_ps[:], yd4[:, kk, D_IN:2 * D_IN], a_tile(ki),
                                     start=(ki == 0), stop=(ki == KT - 1))
            gdz_sc = wk.tile([D_IN, COLS], BF16, name="gdz_sc")
            nc.vector.tensor_mul(gdz_sc[:], gdz_ps[:], dis_bc[0:D_IN, :])
            uzd_ps = mps.tile([D_IN, COLS], F32, tag="uda", name="uzd_ps")
            nc.tensor.matmul(uzd_ps[:], w("wld_z"), gdz_sc[:], start=True, stop=True)
            zcd_t = wk.tile([D_IN, COLS], F32, name="zcd_t")
            nc.scalar.activation(out=zcd_t[:], in_=uzd_ps[:], func=AF.Sigmoid,
                                 bias=b("nbldz"), scale=-1.0)
            gdh_sc = wk.tile([D_IN, COLS], BF16, name="gdh_sc")
            nc.vector.tensor_mul(gdh_sc[:], gdh_ps[:], dis_bc[0:D_IN, :])
            uhd_ps = mps.tile([D_IN, COLS], F32, tag="sp", name="uhd_ps")
            nc.tensor.matmul(uhd_ps[:], w("wld_h"), gdh_sc[:], start=True, stop=True)
            htd_t = wk.tile([D_IN, COLS], F32, name="htd_t")
            nc.scalar.activation(out=htd_t[:], in_=uhd_ps[:], func=AF.Tanh,
                                 bias=b("bldh"))
            prod_t = wk.tile([D_IN, COLS], F32, name="prod_t")
            nc.vector.tensor_mul(prod_t[:], zcd_t[:], htd_t[:])
            rec_t = wk.tile([D_IN, COLS], F32, name="rec_t")
            nc.vector.tensor_scalar_max(rec_t[:], prod_t[:], 0.0)
            nc.scalar.dma_start(out=rec_out[:], in_=rec_t[:])

    nc.compile()
    return nc


def _get_nc():
    if "nc" not in _CACHE:
        _CACHE["nc"] = _build()
    return _CACHE["nc"]


def _eps():
    if "eps" not in _CACHE:
        import jax

        with jax.default_device(jax.devices("cpu")[0]):
            e = jax.random.normal(jax.random.key(42), (N, LAT), jax.numpy.float32)
        _CACHE["eps"] = np.asarray(e)
    return _CACHE["eps"]


def _np(v):
    return np.asarray(v, dtype=np.float32)


def _pack_T(arr_rc, feat):
    # (COLS, P, feat) -> (feat, P*COLS), period-major column blocks, bf16
    a = arr_rc.transpose(1, 2, 0)  # (P, feat, COLS)
    out = np.empty((feat, P * COLS), dtype=BF)
    for p in range(P):
        out[:, p * COLS:(p + 1) * COLS] = a[p].astype(BF)
    return out


def make_in_maps(x, entity_emb, time_emb, params):
    x = _np(x)
    ee = _np(entity_emb)
    te = _np(time_emb)
    p = params
    t1, td = p["t1"], p["td"]
    eps = _eps()

    wc = np.concatenate([_np(t1["Wc_z"]), _np(t1["Wc_h"])], 1)
    wvals = {
        "w_ent": _np(p["W_ent"]), "w_time": _np(p["W_time"]),
        "wc_x": wc[:D_IN], "wc_e": wc[D_IN:D_IN + HID], "wc_t": wc[D_IN + HID:],
        "wl_z": _np(t1["Wl_z"])[:HID], "wl_h": _np(t1["Wl_h"])[:HID],
        "w_mu": _np(p["W_mu"]), "w_lv": _np(p["W_lv"]), "w_dec": _np(p["W_dec"]),
        "wd_cat": np.concatenate([_np(td["Wc_z"]), _np(td["Wc_h"])], 1),
        "wld_z": _np(td["Wl_z"])[:D_IN], "wld_h": _np(td["Wl_h"])[:D_IN],
    }
    wblob = np.zeros((128, WBLOB_COLS), dtype=BF)
    for name, (o, pp, c) in WOFF.items():
        wblob[0:pp, o:o + c] = wvals[name].astype(BF)

    att = _np(p["att1"])
    pr = np.exp(att - att.max())
    pr = (pr / pr.sum()).astype(np.float32)
    bvals = {
        "nblz": -(_np(t1["bc_z"]) @ _np(t1["Wl_z"])[:HID] + _np(t1["bl_z"])).reshape(HID, 1),
        "blh": (_np(t1["bc_h"]) @ _np(t1["Wl_h"])[:HID] + _np(t1["bl_h"])).reshape(HID, 1),
        "b_mu": _np(p["b_mu"]).reshape(LAT, 1),
        "b_lv": _np(p["b_lv"]).reshape(LAT, 1),
        "b_dec": _np(p["b_dec"]).reshape(HID, 1),
        "nbldz": -(_np(td["bc_z"]) @ _np(td["Wl_z"])[:D_IN] + _np(td["bl_z"])).reshape(D_IN, 1),
        "bldh": (_np(td["bc_h"]) @ _np(td["Wl_h"])[:D_IN] + _np(td["bl_h"])).reshape(D_IN, 1),
        "probs": np.broadcast_to(pr, (128, P)),
        "id4": np.eye(4, dtype=np.float32),
        "id32": np.eye(32, dtype=np.float32),
    }
    bblob = np.zeros((128, BBLOB_COLS), dtype=np.float32)
    for name, (o, pp, c) in BOFF.items():
        bblob[0:pp, o:o + c] = bvals[name].astype(np.float32)

    es_full = _np(p["edge_score"])
    in_maps = []
    for c in range(NCORES):
        rc = slice(c * COLS, (c + 1) * COLS)
        blk = es_full[:, rc]  # (4096, 512)
        esc_tiled = np.ascontiguousarray(
            blk.reshape(KT, 128, COLS).transpose(1, 0, 2).reshape(128, KT * COLS))
        in_maps.append({
            "wblob": wblob, "bblob": bblob, "esc": esc_tiled,
            "xTp": _pack_T(x[rc], D_IN),
            "eeTp": _pack_T(ee[rc], D_EMB),
            "teTp": _pack_T(te[rc], D_EMB),
            "epsT": np.ascontiguousarray(eps[rc].T),
        })
    return in_maps


def assemble(results):
    a_blocks, mu_blocks, lv_blocks, rec_blocks = [], [], [], []
    for c in range(NCORES):
        r = results[c]
        a_blocks.append(
            r["a_out"].reshape(128, KT, COLS).transpose(1, 0, 2).reshape(N, COLS))
        mu_blocks.append(r["mu_out"].T)
        lv_blocks.append(r["lv_out"].T)
        rec_blocks.append(r["rec_out"].T)
    A = np.concatenate(a_blocks, axis=1)
    mu = np.concatenate(mu_blocks, axis=0)
    lv = np.concatenate(lv_blocks, axis=0)
    rec = np.concatenate(rec_blocks, axis=0)
    return rec, mu, lv, A


def kernel(x, entity_emb, time_emb, num_nodes, params):
    nc = _get_nc()
    in_maps = make_in_maps(x, entity_emb, time_emb, params)
    res = run_bass_kernel_spmd(nc, in_maps, list(range(NCORES)))
    return assemble(res.results)
